# revision 1
# baseline (speedup 1.0000x reference)
"""Two-way cross-attention block (SuperGlue-style) on 8 trn2 NeuronCores.

Sharding: attention is sharded by head (8 heads -> 8 cores); the MLP /
conv1x1(Wm,W1,W2) + instance-norm part is sharded by sequence position
(2048 -> 8 chunks of 256).  Cross-core traffic per block: one AllToAll of
the per-head attention outputs (1 MB/rank) and an AllReduce of
instance-norm sufficient stats (16 KB).  Block 2 attends to the updated
source; its k/v are projected *sender-side* from each core's resident
src_out chunk and shipped with a single merged AllToAll (2 MB/rank),
which is far cheaper than AllGathering src_out and re-projecting.

All compute fp32.  Exact-math rewrites vs the reference:
  * v-projection bias applied after softmax normalization (rows sum to 1).
  * W1's conv bias cancels inside the affine-free InstanceNorm -> dropped.
  * softmax without max-subtraction (scores are small, safe in fp32).
"""

import sys

if "/opt/trn_rl_repo" not in sys.path:
    sys.path.insert(0, "/opt/trn_rl_repo")

import numpy as np

import concourse.bass as bass
import concourse.mybir as mybir
import concourse.tile as tile
from concourse import bacc
from concourse.bass_utils import run_bass_kernel_spmd

F32 = mybir.dt.float32
AF = mybir.ActivationFunctionType
ALU = mybir.AluOpType
AX = mybir.AxisListType

B = 2        # batch
F = 512      # feature dim
H = 8        # heads
D = 64       # head depth
N = 2048     # sequence length
NL = N // 8  # per-core position chunk (256)
NB = B * NL  # phase-C moving width (both batches concatenated)
NCORES = 8
EPS = 1e-5
RG = [list(range(NCORES))]

_CACHE = {}


def build_nc(reps=1):
    nc = bacc.Bacc("TRN2", target_bir_lowering=False, num_devices=NCORES)

    # ---------------- external I/O (per core) ----------------
    src = nc.dram_tensor("src", [B, F, N], F32, kind="ExternalInput")
    tgt = nc.dram_tensor("tgt", [B, F, N], F32, kind="ExternalInput")
    x1c = nc.dram_tensor("x1c", [B, F, NL], F32, kind="ExternalInput")
    x2c = nc.dram_tensor("x2c", [B, F, NL], F32, kind="ExternalInput")
    wq = nc.dram_tensor("wq", [F, D], F32, kind="ExternalInput")
    wk = nc.dram_tensor("wk", [F, D], F32, kind="ExternalInput")
    wv = nc.dram_tensor("wv", [F, D], F32, kind="ExternalInput")
    bqk = nc.dram_tensor("bqk", [D, 2], F32, kind="ExternalInput")
    bv = nc.dram_tensor("bv", [D, 1], F32, kind="ExternalInput")
    wmT = nc.dram_tensor("wmT", [F, F], F32, kind="ExternalInput")
    bm = nc.dram_tensor("bm", [F, 1], F32, kind="ExternalInput")
    w1T = nc.dram_tensor("w1T", [2 * F, 2 * F], F32, kind="ExternalInput")
    w2T = nc.dram_tensor("w2T", [2 * F, F], F32, kind="ExternalInput")
    b2 = nc.dram_tensor("b2", [F, 1], F32, kind="ExternalInput")
    # block-2 sender-side projection weights (output channels head-major)
    wk2T = nc.dram_tensor("wk2T", [F, F], F32, kind="ExternalInput")
    wv2T = nc.dram_tensor("wv2T", [F, F], F32, kind="ExternalInput")
    bk2 = nc.dram_tensor("bk2", [F, 1], F32, kind="ExternalInput")

    src_out_c = nc.dram_tensor("src_out_c", [B, F, NL], F32, kind="ExternalOutput")
    tgt_out_c = nc.dram_tensor("tgt_out_c", [B, F, NL], F32, kind="ExternalOutput")

    # ---------------- internal DRAM (collectives) ----------------
    cc_a_in = [nc.dram_tensor(f"cc_a_in{i}", [NCORES, D, B, NL], F32)
               for i in range(2)]
    a2a_a = [nc.dram_tensor(f"a2a_a{i}", [NCORES, D, B, NL], F32)
             for i in range(2)]
    cc_st_in = [nc.dram_tensor(f"cc_st_in{i}", [128, 32], F32) for i in range(2)]
    ar_st = [nc.dram_tensor(f"ar_st{i}", [128, 32], F32, addr_space="Shared")
             for i in range(2)]
    # merged k/v AllToAll for block 2: slot h = [k rows of head h | vT cols]
    cc_kv_in = nc.dram_tensor("cc_kv_in", [NCORES, 2, B, D, NL], F32)
    a2a_kv = nc.dram_tensor("a2a_kv", [NCORES, 2, B, D, NL], F32)

    with tile.TileContext(nc) as tc, bass.ExitStack() as ctx:
        # ---------- persistent tiles ----------
        wp = ctx.enter_context(tc.tile_pool(name="weights", bufs=1))
        wq_sb = wp.tile([128, 4, D], F32, tag="wq")
        wk_sb = wp.tile([128, 4, D], F32, tag="wk")
        wv_sb = wp.tile([128, 4, D], F32, tag="wv")
        bqk_sb = wp.tile([D, 2], F32, tag="bqk")
        bv_sb = wp.tile([D, 1], F32, tag="bv")
        wm_sb = wp.tile([128, 4, F], F32, tag="wm")
        bm_sb = wp.tile([128, 4], F32, tag="bm")
        w1_sb = wp.tile([128, 8, 2 * F], F32, tag="w1")
        w2_sb = wp.tile([128, 8, F], F32, tag="w2")
        b2_sb = wp.tile([128, 4], F32, tag="b2")
        wk2_sb = wp.tile([128, 4, F], F32, tag="wk2")
        wv2_sb = wp.tile([128, 4, F], F32, tag="wv2")
        bk2_sb = wp.tile([128, 4], F32, tag="bk2")
        ones1 = wp.tile([1, D], F32, tag="ones1")
        nc.vector.memset(ones1[:], 1.0)

        for t, d_ in ((wq_sb, wq), (wk_sb, wk), (wv_sb, wv)):
            nc.sync.dma_start(out=t[:], in_=d_[:].rearrange("(t p) d -> p t d", p=128))
        nc.sync.dma_start(out=bqk_sb[:], in_=bqk[:])
        nc.sync.dma_start(out=bv_sb[:], in_=bv[:])
        for t, d_ in ((wm_sb, wmT), (w1_sb, w1T), (w2_sb, w2T),
                      (wk2_sb, wk2T), (wv2_sb, wv2T)):
            nc.sync.dma_start(out=t[:], in_=d_[:].rearrange("(t p) o -> p t o", p=128))
        for t, d_ in ((bm_sb, bm), (b2_sb, b2), (bk2_sb, bk2)):
            nc.sync.dma_start(out=t[:],
                              in_=d_[:].rearrange("(t p) one -> p (t one)", p=128))

        def emit_q_proj(qsrc_dram, q_sb, name):
            """q_sb[d, b, n] = (Wq_h @ qsrc + bq_h), streamed over f-tiles."""
            with tc.tile_pool(name=f"tb{name}", bufs=5) as tbp, \
                 tc.tile_pool(name=f"psQ{name}", bufs=4, space="PSUM") as psQ:
                for b in range(B):
                    qtiles = []
                    for kf in range(4):
                        t = tbp.tile([128, N], F32, tag="kv")
                        nc.sync.dma_start(out=t[:],
                                          in_=qsrc_dram[b, kf * 128:(kf + 1) * 128, :])
                        qtiles.append(t)
                    for nt in range(4):
                        ps = psQ.tile([D, 512], F32, tag="qps")
                        for kf in range(4):
                            nc.tensor.matmul(ps[:], wq_sb[:, kf, :],
                                             qtiles[kf][:, nt * 512:(nt + 1) * 512],
                                             start=(kf == 0), stop=(kf == 3))
                        nc.vector.tensor_scalar(
                            q_sb[:, b, nt * 512:(nt + 1) * 512],
                            ps[:], bqk_sb[:, 0:1], None, ALU.add)

        def emit_kv_proj_block0(k_sb, vt_sb):
            """Block-0 k and vT projections from tgt (streamed)."""
            with tc.tile_pool(name="tbKV0", bufs=5) as tbp, \
                 tc.tile_pool(name="psK0", bufs=4, space="PSUM") as psA, \
                 tc.tile_pool(name="psV0", bufs=4, space="PSUM") as psVT:
                for b in range(B):
                    tiles = []
                    for kf in range(4):
                        t = tbp.tile([128, N], F32, tag="kv")
                        nc.sync.dma_start(out=t[:],
                                          in_=tgt[b, kf * 128:(kf + 1) * 128, :])
                        tiles.append(t)
                    for nt in range(4):
                        ps = psA.tile([D, 512], F32, tag="kps")
                        for kf in range(4):
                            nc.tensor.matmul(ps[:], wk_sb[:, kf, :],
                                             tiles[kf][:, nt * 512:(nt + 1) * 512],
                                             start=(kf == 0), stop=(kf == 3))
                        nc.vector.tensor_scalar(
                            k_sb[:, b, nt * 512:(nt + 1) * 512],
                            ps[:], bqk_sb[:, 1:2], None, ALU.add)
                    for mi in range(16):
                        ps = psVT.tile([128, D], F32, tag="vtps")
                        for kf in range(4):
                            nc.tensor.matmul(ps[:],
                                             tiles[kf][:, mi * 128:(mi + 1) * 128],
                                             wv_sb[:, kf, :],
                                             start=(kf == 0), stop=(kf == 3))
                        nc.vector.tensor_copy(vt_sb[:, b, mi, 0:D], ps[:])

        def emit_attention(block, q_sb, k_sb, vt_sb, araw, pp, dnp, psS, psBC,
                           psPV):
            for b in range(B):
                for nb in range(4):
                    nsl = slice(nb * 512, (nb + 1) * 512)
                    p_sb = pp.tile([128, 16, 512], F32, tag="p")
                    for g in range(8):
                        ps = psS.tile([128, 2, 512], F32, tag="sps")
                        for j in range(2):
                            mi = 2 * g + j
                            nc.tensor.matmul(
                                ps[:, j, :],
                                k_sb[:, b, mi * 128:(mi + 1) * 128],
                                q_sb[:, b, nsl], start=True, stop=True)
                        nc.scalar.activation(p_sb[:, 2 * g:2 * g + 2, :], ps[:],
                                             AF.Exp, scale=float(1.0 / np.sqrt(D)))
                    pv = psPV.tile([D + 1, 512], F32, tag="pvps")
                    for mi in range(16):
                        nc.tensor.matmul(pv[:], vt_sb[:, b, mi, :],
                                         p_sb[:, mi, :],
                                         start=(mi == 0), stop=(mi == 15))
                    nc.vector.tensor_copy(araw[:, b, nsl], pv[:])
                # normalize: a = araw[0:64]/araw[64] + bv; den row is bounced
                # to partition 0 by a small SBUF->SBUF DMA, the broadcast is
                # a K=1 ones-matmul on PE (gpsimd broadcast is ~11us/op).
                rden = dnp.tile([1, N], F32, tag="rden")
                nc.sync.dma_start(out=rden[:], in_=araw[D:D + 1, b, :])
                nc.vector.reciprocal(rden[:], rden[:])
                for nb in range(4):
                    nsl = slice(nb * 512, (nb + 1) * 512)
                    bc = psBC.tile([D, 512], F32, tag="bc")
                    nc.tensor.matmul(bc[:], ones1[:], rden[:, nsl],
                                     start=True, stop=True)
                    nc.vector.tensor_mul(araw[0:D, b, nsl], araw[0:D, b, nsl],
                                         bc[:])
            nc.vector.tensor_scalar(araw[0:D, :, :], araw[0:D, :, :], bv_sb[:],
                                    None, ALU.add)
            nc.sync.dma_start(
                out=cc_a_in[block][:].rearrange("c d b n -> d b c n"),
                in_=araw[0:D, :, :].rearrange("d b (c n) -> d b c n", c=NCORES))
            nc.gpsimd.collective_compute(
                "AllToAll", ALU.bypass, replica_groups=RG,
                ins=[cc_a_in[block][:]], outs=[a2a_a[block][:]])

        def emit_mlp(block, x_chunk_dram, out_dram, send_kv):
            with tc.tile_pool(name=f"xC{block}", bufs=1) as xcp, \
                 tc.tile_pool(name=f"aC{block}", bufs=8) as acp, \
                 tc.tile_pool(name=f"amC{block}", bufs=1) as amp, \
                 tc.tile_pool(name=f"hC{block}", bufs=1) as hp, \
                 tc.tile_pool(name=f"stC{block}", bufs=1) as stp, \
                 tc.tile_pool(name=f"scrC{block}", bufs=2) as scp, \
                 tc.tile_pool(name=f"soC{block}", bufs=1) as sop, \
                 tc.tile_pool(name=f"psC{block}", bufs=6, space="PSUM") as psC:
                x_sb = xcp.tile([128, 4, B, NL], F32, tag="x")
                for b in range(B):
                    nc.sync.dma_start(
                        out=x_sb[:, :, b, :],
                        in_=x_chunk_dram[b].rearrange("(t p) n -> p t n", p=128))
                am_sb = amp.tile([128, 4, B, NL], F32, tag="am")
                h1_sb = hp.tile([128, 8, B, NL], F32, tag="h1")
                stats = stp.tile([128, 32], F32, tag="st")
                atiles = []
                for g in range(4):
                    at = acp.tile([128, B, NL], F32, tag="ach")
                    nc.sync.dma_start(
                        out=at[:],
                        in_=a2a_a[block][2 * g:2 * g + 2, :, :, :].rearrange(
                            "c d b n -> (c d) b n"))
                    atiles.append(at)
                for o in range(4):
                    ps = psC.tile([128, NB], F32, tag="cps")
                    for g in range(4):
                        nc.tensor.matmul(ps[:], wm_sb[:, g, o * 128:(o + 1) * 128],
                                         atiles[g][:],
                                         start=(g == 0), stop=(g == 3))
                    nc.vector.tensor_scalar(am_sb[:, o, :, :], ps[:],
                                            bm_sb[:, o:o + 1], None, ALU.add)
                for o in range(8):
                    ps = psC.tile([128, NB], F32, tag="cps")
                    for g in range(8):
                        rhs = (x_sb[:, g, :, :] if g < 4
                               else am_sb[:, g - 4, :, :])
                        nc.tensor.matmul(ps[:], w1_sb[:, g, o * 128:(o + 1) * 128],
                                         rhs, start=(g == 0), stop=(g == 7))
                    nc.vector.tensor_copy(h1_sb[:, o, :, :], ps[:])
                    for b in range(B):
                        col = b * 8 + o
                        nc.vector.tensor_reduce(stats[:, 2 * col:2 * col + 1],
                                                h1_sb[:, o, b, :], AX.X, ALU.add)
                        # tensor_tensor_reduce crashes this runtime; ACT
                        # Square with accum_out computes the sum of squares.
                        scr = scp.tile([128, NL], F32, tag="sq")
                        nc.scalar.activation(
                            scr[:], h1_sb[:, o, b, :], AF.Square,
                            accum_out=stats[:, 2 * col + 1:2 * col + 2])
                nc.sync.dma_start(out=cc_st_in[block][:], in_=stats[:])
                nc.gpsimd.collective_compute(
                    "AllReduce", ALU.add, replica_groups=RG,
                    ins=[cc_st_in[block][:]], outs=[ar_st[block][:]])
                stg = stp.tile([128, 32], F32, tag="stg")
                nc.sync.dma_start(out=stg[:], in_=ar_st[block][:])
                mean = stp.tile([128, 16], F32, tag="mean")
                vtmp = stp.tile([128, 16], F32, tag="vtmp")
                rstd = stp.tile([128, 16], F32, tag="rstd")
                nbias = stp.tile([128, 16], F32, tag="nbias")
                s1v = stg[:].rearrange("p (c two) -> p c two", two=2)
                nc.vector.tensor_scalar(mean[:], s1v[:, :, 0], 1.0 / N, None,
                                        ALU.mult)
                nc.vector.tensor_scalar(vtmp[:], s1v[:, :, 1], 1.0 / N, None,
                                        ALU.mult)
                nc.vector.tensor_mul(nbias[:], mean[:], mean[:])
                nc.vector.tensor_sub(vtmp[:], vtmp[:], nbias[:])
                nc.vector.tensor_scalar(vtmp[:], vtmp[:], EPS, None, ALU.add)
                nc.scalar.sqrt(vtmp[:], vtmp[:])
                nc.vector.reciprocal(rstd[:], vtmp[:])
                nc.vector.tensor_mul(nbias[:], mean[:], rstd[:])
                nc.vector.tensor_scalar(nbias[:], nbias[:], -1.0, None, ALU.mult)
                # IN + relu applied in place (h1 is dead afterwards)
                hr_sb = h1_sb
                for o in range(8):
                    for b in range(B):
                        col = b * 8 + o
                        nc.scalar.activation(hr_sb[:, o, b, :], h1_sb[:, o, b, :],
                                             AF.Relu, bias=nbias[:, col:col + 1],
                                             scale=rstd[:, col:col + 1])
                so_sb = sop.tile([128, 4, B, NL], F32, tag="so")
                for o in range(4):
                    ps = psC.tile([128, NB], F32, tag="cps")
                    for g in range(8):
                        nc.tensor.matmul(ps[:], w2_sb[:, g, o * 128:(o + 1) * 128],
                                         hr_sb[:, g, :, :],
                                         start=(g == 0), stop=(g == 7))
                    nc.vector.tensor_scalar(so_sb[:, o, :, :], ps[:],
                                            b2_sb[:, o:o + 1], None, ALU.add)
                    nc.vector.tensor_add(so_sb[:, o, :, :], so_sb[:, o, :, :],
                                         x_sb[:, o, :, :])
                    for b in range(B):
                        nc.sync.dma_start(
                            out=out_dram[b, o * 128:(o + 1) * 128, :],
                            in_=so_sb[:, o, b, :])
                if send_kv:
                    # sender-side block-2 k/vT projections from the resident
                    # src_out chunk, then one merged AllToAll.
                    with tc.tile_pool(name="kvS", bufs=1) as kvp, \
                         tc.tile_pool(name="psS2", bufs=2, space="PSUM") as psS2:
                        k2_sb = kvp.tile([128, 4, B, NL], F32, tag="k2")
                        v2_sb = kvp.tile([128, 2, B, F], F32, tag="v2t")
                        for t in range(4):
                            ps = psS2.tile([128, NB], F32, tag="s2ps")
                            for g in range(4):
                                nc.tensor.matmul(
                                    ps[:], wk2_sb[:, g, t * 128:(t + 1) * 128],
                                    so_sb[:, g, :, :],
                                    start=(g == 0), stop=(g == 3))
                            nc.vector.tensor_scalar(k2_sb[:, t, :, :], ps[:],
                                                    bk2_sb[:, t:t + 1], None,
                                                    ALU.add)
                        for mt in range(2):
                            for b in range(B):
                                ps = psS2.tile([128, F], F32, tag="s2ps")
                                for g in range(4):
                                    nc.tensor.matmul(
                                        ps[:],
                                        so_sb[:, g, b,
                                              mt * 128:(mt + 1) * 128],
                                        wv2_sb[:, g, :],
                                        start=(g == 0), stop=(g == 3))
                                nc.vector.tensor_copy(v2_sb[:, mt, b, :], ps[:])
                        for h in range(NCORES):
                            t, r = h // 2, (h % 2) * D
                            nc.sync.dma_start(
                                out=cc_kv_in[h, 0].rearrange("b p n -> p b n"),
                                in_=k2_sb[r:r + D, t, :, :])
                            for b in range(B):
                                nc.sync.dma_start(
                                    out=cc_kv_in[h, 1, b].rearrange(
                                        "d n -> (d n)").rearrange(
                                        "(mt p d) -> p mt d", mt=2, p=128),
                                    in_=v2_sb[:, :, b, h * D:(h + 1) * D])
                        nc.gpsimd.collective_compute(
                            "AllToAll", ALU.bypass, replica_groups=RG,
                            ins=[cc_kv_in[:]], outs=[a2a_kv[:]])

        def emit_kv_assembly_block1(k_sb, vt_sb):
            for b in range(B):
                nc.sync.dma_start(
                    out=k_sb[:, b, :].rearrange("d (c n) -> d c n", c=NCORES),
                    in_=a2a_kv[:, 0, b, :, :].rearrange("c d n -> d c n"))
                for c in range(NCORES):
                    nc.sync.dma_start(
                        out=vt_sb[:, b, 2 * c:2 * c + 2, 0:D],
                        in_=a2a_kv[c, 1, b, :, :].rearrange(
                            "d n -> (d n)").rearrange(
                            "(mt p d) -> p mt d", mt=2, p=128))

        for rp in range(reps):
            # ---- block 0 ----
            with tc.tile_pool(name=f"qk0_{rp}", bufs=1) as qkp:
                q_sb = qkp.tile([D, B, N], F32, tag="q")
                k_sb = qkp.tile([D, B, N], F32, tag="k")
                vt_sb = qkp.tile([128, B, 16, D + 1], F32, tag="vt")
                araw = qkp.tile([D + 1, B, N], F32, tag="araw")
                nc.vector.memset(vt_sb[:, :, :, D], 1.0)
                emit_kv_proj_block0(k_sb, vt_sb)
                emit_q_proj(src, q_sb, f"q0_{rp}")
                with tc.tile_pool(name=f"pB0_{rp}", bufs=2) as pp, \
                     tc.tile_pool(name=f"dB0_{rp}", bufs=1) as dnp, \
                     tc.tile_pool(name=f"psS0_{rp}", bufs=2, space="PSUM") as psS, \
                     tc.tile_pool(name=f"psBC0_{rp}", bufs=2, space="PSUM") as psBC, \
                     tc.tile_pool(name=f"psPV0_{rp}", bufs=2, space="PSUM") as psPV:
                    emit_attention(0, q_sb, k_sb, vt_sb, araw, pp, dnp, psS,
                                   psBC, psPV)
            # ---- block 1 ----  (pool opened early so q2 shares it; the
            # q2 projection fills the A2A(a0) + AR(st0) stalls)
            with tc.tile_pool(name=f"qk1_{rp}", bufs=1) as qkp:
                q2_sb = qkp.tile([D, B, N], F32, tag="q2")
                emit_q_proj(tgt, q2_sb, f"q1_{rp}")
                emit_mlp(0, x1c, src_out_c, send_kv=True)
                k_sb = qkp.tile([D, B, N], F32, tag="k")
                vt_sb = qkp.tile([128, B, 16, D + 1], F32, tag="vt")
                araw = qkp.tile([D + 1, B, N], F32, tag="araw")
                nc.vector.memset(vt_sb[:, :, :, D], 1.0)
                emit_kv_assembly_block1(k_sb, vt_sb)
                with tc.tile_pool(name=f"pB1_{rp}", bufs=2) as pp, \
                     tc.tile_pool(name=f"dB1_{rp}", bufs=1) as dnp, \
                     tc.tile_pool(name=f"psS1_{rp}", bufs=2, space="PSUM") as psS, \
                     tc.tile_pool(name=f"psBC1_{rp}", bufs=2, space="PSUM") as psBC, \
                     tc.tile_pool(name=f"psPV1_{rp}", bufs=2, space="PSUM") as psPV:
                    emit_attention(1, q2_sb, k_sb, vt_sb, araw, pp, dnp, psS,
                                   psBC, psPV)
            emit_mlp(1, x2c, tgt_out_c, send_kv=False)

    nc.finalize()
    return nc


def _prep_inputs(src, tgt, Wq, bq, Wk, bk, Wv, bv, Wm, bm, W1, b1, W2, b2):
    """Host-side slicing/permutation into the per-core in_maps."""
    src = np.ascontiguousarray(src, np.float32)
    tgt = np.ascontiguousarray(tgt, np.float32)
    perm = np.arange(F).reshape(D, H).T.reshape(F)  # f' = h*64+d -> f = d*8+h
    wm_perm = np.ascontiguousarray(
        np.asarray(Wm).reshape(F, D, H).transpose(2, 1, 0).reshape(F, F),
        np.float32)
    w1t = np.ascontiguousarray(np.asarray(W1).T, np.float32)
    w2t = np.ascontiguousarray(np.asarray(W2).T, np.float32)
    wk2t = np.ascontiguousarray(np.asarray(Wk)[perm, :].T, np.float32)
    wv2t = np.ascontiguousarray(np.asarray(Wv)[perm, :].T, np.float32)
    bk2 = np.ascontiguousarray(np.asarray(bk)[perm].reshape(F, 1), np.float32)
    bm_c = np.ascontiguousarray(np.asarray(bm).reshape(F, 1), np.float32)
    b2_c = np.ascontiguousarray(np.asarray(b2).reshape(F, 1), np.float32)
    in_maps = []
    for h in range(NCORES):
        f_list = np.arange(D) * H + h
        bqk_h = np.stack([np.asarray(bq)[f_list], np.asarray(bk)[f_list]],
                         axis=1).astype(np.float32)
        sl = slice(h * NL, (h + 1) * NL)
        in_maps.append({
            "src": src,
            "tgt": tgt,
            "x1c": np.ascontiguousarray(src[:, :, sl]),
            "x2c": np.ascontiguousarray(tgt[:, :, sl]),
            "wq": np.ascontiguousarray(np.asarray(Wq)[f_list, :].T, np.float32),
            "wk": np.ascontiguousarray(np.asarray(Wk)[f_list, :].T, np.float32),
            "wv": np.ascontiguousarray(np.asarray(Wv)[f_list, :].T, np.float32),
            "bqk": np.ascontiguousarray(bqk_h),
            "bv": np.ascontiguousarray(
                np.asarray(bv)[f_list].reshape(D, 1), np.float32),
            "wmT": wm_perm,
            "bm": bm_c,
            "w1T": w1t,
            "w2T": w2t,
            "b2": b2_c,
            "wk2T": wk2t,
            "wv2T": wv2t,
            "bk2": bk2,
        })
    return in_maps


def kernel(**inputs):
    if "nc" not in _CACHE:
        _CACHE["nc"] = build_nc()
    nc = _CACHE["nc"]
    in_maps = _prep_inputs(**inputs)
    res = run_bass_kernel_spmd(nc, in_maps, list(range(NCORES)))
    src_out = np.concatenate(
        [res.results[c]["src_out_c"] for c in range(NCORES)], axis=2)
    tgt_out = np.concatenate(
        [res.results[c]["tgt_out_c"] for c in range(NCORES)], axis=2)
    return (src_out, tgt_out)



# revision 11
# speedup vs baseline: 445.3958x; 445.3958x over previous
"""Two-way cross-attention block (SuperGlue-style) on 8 trn2 NeuronCores.

Sharding: attention is sharded by head (8 heads -> 8 cores); the MLP /
conv1x1(Wm,W1,W2) + instance-norm part is sharded by sequence position
(2048 -> 8 chunks of 256).  All cross-core collectives are split per
batch element and pipelined against the other batch's compute: the
attention-out AllToAll for b=0 flies while b=1's scores/softmax run, and
block-2's merged k/v AllToAll for b=1 is hidden under block-2 attention
on b=0.  Instance-norm sufficient stats use AllGather + a local 8-way
sum (cheaper than AllReduce in wall-clock, and only 16 KB/rank).

All matmul operands are bf16 (PE runs 4x faster than fp32; PSUM
accumulation stays fp32).  Exact-math rewrites vs the reference:
  * v-projection bias applied after softmax normalization (rows sum to 1).
  * k-projection bias dropped entirely: it shifts every score in a softmax
    row by the same q.bk constant, which cancels in the softmax.
  * W1's conv bias cancels inside the affine-free InstanceNorm -> dropped.
  * softmax without max-subtraction (scores are small, safe in fp32).
"""

import sys

if "/opt/trn_rl_repo" not in sys.path:
    sys.path.insert(0, "/opt/trn_rl_repo")

import numpy as np

import concourse.bass as bass
import concourse.mybir as mybir
import concourse.tile as tile
from concourse import bacc
from concourse.bass_utils import run_bass_kernel_spmd

F32 = mybir.dt.float32
BF16 = mybir.dt.bfloat16
NPBF16 = mybir.dt.np(mybir.dt.bfloat16)
AF = mybir.ActivationFunctionType
ALU = mybir.AluOpType
AX = mybir.AxisListType

B = 2        # batch
F = 512      # feature dim
H = 8        # heads
D = 64       # head depth
N = 2048     # sequence length
NL = N // 8  # per-core position chunk (256)
NCORES = 8
EPS = 1e-5
RG = [list(range(NCORES))]

_CACHE = {}


def build_nc(reps=1):
    nc = bacc.Bacc("TRN2", target_bir_lowering=False, num_devices=NCORES)

    # ---------------- external I/O (per core) ----------------
    src = nc.dram_tensor("src", [B, F, N], BF16, kind="ExternalInput")
    tgt = nc.dram_tensor("tgt", [B, F, N], BF16, kind="ExternalInput")
    x1c = nc.dram_tensor("x1c", [B, F, NL], BF16, kind="ExternalInput")
    x2c = nc.dram_tensor("x2c", [B, F, NL], BF16, kind="ExternalInput")
    wq = nc.dram_tensor("wq", [F, D], BF16, kind="ExternalInput")
    wk = nc.dram_tensor("wk", [F, D], BF16, kind="ExternalInput")
    wv = nc.dram_tensor("wv", [F, D], BF16, kind="ExternalInput")
    bq = nc.dram_tensor("bq", [D, 1], F32, kind="ExternalInput")
    bv = nc.dram_tensor("bv", [D, 1], F32, kind="ExternalInput")
    wmT = nc.dram_tensor("wmT", [F, F], BF16, kind="ExternalInput")
    bm = nc.dram_tensor("bm", [F, 1], F32, kind="ExternalInput")
    w1T = nc.dram_tensor("w1T", [2 * F, 2 * F], BF16, kind="ExternalInput")
    w2T = nc.dram_tensor("w2T", [2 * F, F], BF16, kind="ExternalInput")
    b2 = nc.dram_tensor("b2", [F, 1], F32, kind="ExternalInput")
    # block-2 sender-side projection weights (output channels head-major)
    wk2T = nc.dram_tensor("wk2T", [F, F], BF16, kind="ExternalInput")
    wv2T = nc.dram_tensor("wv2T", [F, F], BF16, kind="ExternalInput")

    src_out_c = nc.dram_tensor("src_out_c", [B, F, NL], F32, kind="ExternalOutput")
    tgt_out_c = nc.dram_tensor("tgt_out_c", [B, F, NL], F32, kind="ExternalOutput")

    # ------------- internal DRAM (per-batch-split collectives) -------------
    cc_a_in = [[nc.dram_tensor(f"cc_a_in{i}_{b}", [NCORES, D, NL], BF16)
                for b in range(B)] for i in range(2)]
    a2a_a = [[nc.dram_tensor(f"a2a_a{i}_{b}", [NCORES, D, NL], BF16)
              for b in range(B)] for i in range(2)]
    cc_st_in = [[nc.dram_tensor(f"cc_st_in{i}_{b}", [128, 16], F32)
                 for b in range(B)] for i in range(2)]
    ag_st = [[nc.dram_tensor(f"ag_st{i}_{b}", [NCORES, 128, 16], F32)
              for b in range(B)] for i in range(2)]
    # merged k/v AllToAll for block 2: slot h = [k rows of head h | vT cols]
    cc_kv_in = [nc.dram_tensor(f"cc_kv_in{b}", [NCORES, 2, D, NL], BF16)
                for b in range(B)]
    a2a_kv = [nc.dram_tensor(f"a2a_kv{b}", [NCORES, 2, D, NL], BF16)
              for b in range(B)]

    with tile.TileContext(nc) as tc, bass.ExitStack() as ctx:
        # ---------- persistent tiles ----------
        wp = ctx.enter_context(tc.tile_pool(name="weights", bufs=1))
        wq_sb = wp.tile([128, 4, D], BF16, tag="wq")
        wk_sb = wp.tile([128, 4, D], BF16, tag="wk")
        wv_sb = wp.tile([128, 4, D], BF16, tag="wv")
        bq_sb = wp.tile([D, 1], F32, tag="bq")
        bv_sb = wp.tile([D, 1], F32, tag="bv")
        wm_sb = wp.tile([128, 4, F], BF16, tag="wm")
        bm_sb = wp.tile([128, 4], F32, tag="bm")
        w1_sb = wp.tile([128, 8, 2 * F], BF16, tag="w1")
        w2_sb = wp.tile([128, 8, F], BF16, tag="w2")
        b2_sb = wp.tile([128, 4], F32, tag="b2")
        wk2_sb = wp.tile([128, 4, F], BF16, tag="wk2")
        wv2_sb = wp.tile([128, 4, F], BF16, tag="wv2")
        ones1 = wp.tile([1, D], BF16, tag="ones1")
        nc.vector.memset(ones1[:], 1.0)

        for t, d_ in ((wq_sb, wq), (wk_sb, wk), (wv_sb, wv)):
            nc.sync.dma_start(out=t[:], in_=d_[:].rearrange("(t p) d -> p t d", p=128))
        nc.sync.dma_start(out=bq_sb[:], in_=bq[:])
        nc.sync.dma_start(out=bv_sb[:], in_=bv[:])

        def emit_heavy_weight_loads():
            # emitted after the first src/tgt stream DMAs: these 2.2 MB of
            # MLP weights aren't needed until ~100us in, and issuing them
            # first would stall the projection matmuls behind them.
            for t, d_ in ((wm_sb, wmT), (w1_sb, w1T), (w2_sb, w2T),
                          (wk2_sb, wk2T), (wv2_sb, wv2T)):
                nc.sync.dma_start(out=t[:],
                                  in_=d_[:].rearrange("(t p) o -> p t o", p=128))
            for t, d_ in ((bm_sb, bm), (b2_sb, b2)):
                nc.sync.dma_start(
                    out=t[:], in_=d_[:].rearrange("(t p) one -> p (t one)", p=128))

        def emit_q_proj(qsrc_dram, q_sb, name):
            """q_sb[d, b, n] = (Wq_h @ qsrc + bq_h), streamed over f-tiles."""
            with tc.tile_pool(name=f"tb{name}", bufs=5) as tbp, \
                 tc.tile_pool(name=f"psQ{name}", bufs=4, space="PSUM") as psQ:
                for b in range(B):
                    qtiles = []
                    for kf in range(4):
                        t = tbp.tile([128, N], BF16, tag="kv")
                        nc.sync.dma_start(out=t[:],
                                          in_=qsrc_dram[b, kf * 128:(kf + 1) * 128, :])
                        qtiles.append(t)
                    for nt in range(4):
                        ps = psQ.tile([D, 512], F32, tag="qps")
                        for kf in range(4):
                            nc.tensor.matmul(ps[:], wq_sb[:, kf, :],
                                             qtiles[kf][:, nt * 512:(nt + 1) * 512],
                                             start=(kf == 0), stop=(kf == 3))
                        nc.vector.tensor_scalar(
                            q_sb[:, b, nt * 512:(nt + 1) * 512],
                            ps[:], bq_sb[:], None, ALU.add)

        def emit_kvq_proj_block0(k_sb, vt_sb, q2_sb):
            """Block-0 k and vT plus block-1 q projections, one tgt stream."""
            with tc.tile_pool(name="tbKV0", bufs=5) as tbp, \
                 tc.tile_pool(name="psK0", bufs=2, space="PSUM") as psA, \
                 tc.tile_pool(name="psQ2", bufs=2, space="PSUM") as psQ2, \
                 tc.tile_pool(name="psV0", bufs=2, space="PSUM") as psVT:
                for b in range(B):
                    tiles = []
                    for kf in range(4):
                        t = tbp.tile([128, N], BF16, tag="kv")
                        nc.sync.dma_start(out=t[:],
                                          in_=tgt[b, kf * 128:(kf + 1) * 128, :])
                        tiles.append(t)
                    for nt in range(4):
                        ps = psA.tile([D, 512], F32, tag="kps")
                        for kf in range(4):
                            nc.tensor.matmul(ps[:], wk_sb[:, kf, :],
                                             tiles[kf][:, nt * 512:(nt + 1) * 512],
                                             start=(kf == 0), stop=(kf == 3))
                        nc.vector.tensor_copy(
                            k_sb[:, b, nt * 512:(nt + 1) * 512], ps[:])
                        ps2 = psQ2.tile([D, 512], F32, tag="q2ps")
                        for kf in range(4):
                            nc.tensor.matmul(ps2[:], wq_sb[:, kf, :],
                                             tiles[kf][:, nt * 512:(nt + 1) * 512],
                                             start=(kf == 0), stop=(kf == 3))
                        nc.vector.tensor_scalar(
                            q2_sb[:, b, nt * 512:(nt + 1) * 512],
                            ps2[:], bq_sb[:], None, ALU.add)
                    for mg in range(4):
                        ps = psVT.tile([128, 4, D], F32, tag="vtps")
                        for j in range(4):
                            mi = 4 * mg + j
                            for kf in range(4):
                                nc.tensor.matmul(
                                    ps[:, j, :],
                                    tiles[kf][:, mi * 128:(mi + 1) * 128],
                                    wv_sb[:, kf, :],
                                    start=(kf == 0), stop=(kf == 3))
                        nc.vector.tensor_copy(
                            vt_sb[:, b, 4 * mg:4 * mg + 4, 0:D], ps[:])

        def emit_attention_b(block, b, q_sb, k_sb, vt_sb, araw, a_sb, pp, dnp,
                             psS, psBC, psPV):
            """One batch element of attention + its per-batch AllToAll."""
            for nb in range(4):
                nsl = slice(nb * 512, (nb + 1) * 512)
                p_sb = pp.tile([128, 16, 512], BF16, tag="p")
                for g in range(8):
                    ps = psS.tile([128, 2, 512], F32, tag="sps")
                    for j in range(2):
                        mi = 2 * g + j
                        nc.tensor.matmul(
                            ps[:, j, :],
                            k_sb[:, b, mi * 128:(mi + 1) * 128],
                            q_sb[:, b, nsl], start=True, stop=True)
                    nc.scalar.activation(p_sb[:, 2 * g:2 * g + 2, :], ps[:],
                                         AF.Exp, scale=float(1.0 / np.sqrt(D)))
                pv = psPV.tile([D + 1, 512], F32, tag="pvps")
                for mi in range(16):
                    nc.tensor.matmul(pv[:], vt_sb[:, b, mi, :],
                                     p_sb[:, mi, :],
                                     start=(mi == 0), stop=(mi == 15))
                nc.vector.tensor_copy(araw[:, b, nsl], pv[:])
            # normalize: a = araw[0:64]/araw[64] + bv; den row is bounced
            # to partition 0 by a small SBUF->SBUF DMA, the broadcast is
            # a K=1 ones-matmul on PE (gpsimd broadcast is ~11us/op).
            rden = dnp.tile([1, N], F32, tag="rden")
            rdenb = dnp.tile([1, N], BF16, tag="rdenb")
            nc.sync.dma_start(out=rden[:], in_=araw[D:D + 1, b, :])
            nc.vector.reciprocal(rden[:], rden[:])
            nc.vector.tensor_copy(rdenb[:], rden[:])
            for nb in range(4):
                nsl = slice(nb * 512, (nb + 1) * 512)
                bc = psBC.tile([D, 512], F32, tag="bc")
                nc.tensor.matmul(bc[:], ones1[:], rdenb[:, nsl],
                                 start=True, stop=True)
                nc.vector.tensor_mul(a_sb[:, b, nsl], araw[0:D, b, nsl],
                                     bc[:])
            nc.vector.tensor_scalar(a_sb[:, b, :], a_sb[:, b, :], bv_sb[:],
                                    None, ALU.add)
            nc.sync.dma_start(
                out=cc_a_in[block][b][:].rearrange("c d n -> d c n"),
                in_=a_sb[:, b, :].rearrange("d (c n) -> d c n", c=NCORES))
            nc.gpsimd.collective_compute(
                "AllToAll", ALU.bypass, replica_groups=RG,
                ins=[cc_a_in[block][b][:]], outs=[a2a_a[block][b][:]])

        def emit_mlp(block, x_chunk_dram, out_dram, send_kv):
            with tc.tile_pool(name=f"xC{block}", bufs=1) as xcp, \
                 tc.tile_pool(name=f"aC{block}", bufs=8) as acp, \
                 tc.tile_pool(name=f"amC{block}", bufs=1) as amp, \
                 tc.tile_pool(name=f"hC{block}", bufs=1) as hp, \
                 tc.tile_pool(name=f"stC{block}", bufs=1) as stp, \
                 tc.tile_pool(name=f"scrC{block}", bufs=2) as scp, \
                 tc.tile_pool(name=f"soC{block}", bufs=1) as sop, \
                 tc.tile_pool(name=f"psC{block}", bufs=6, space="PSUM") as psC:
                x_sb = xcp.tile([128, 4, B, NL], BF16, tag="x")
                for b in range(B):
                    nc.sync.dma_start(
                        out=x_sb[:, :, b, :],
                        in_=x_chunk_dram[b].rearrange("(t p) n -> p t n", p=128))
                am_sb = amp.tile([128, 4, B, NL], BF16, tag="am")
                h1_sb = hp.tile([128, 8, B, NL], BF16, tag="h1")
                hr_sb = hp.tile([128, 8, B, NL], BF16, tag="hr")
                # per-batch front: Wm + W1 + stats + that batch's stats
                # AllGather, so b=0's AG flies during b=1's Wm/W1 compute.
                for b in range(B):
                    stats = stp.tile([128, 16], F32, tag=f"st{b}")
                    atiles = []
                    for g in range(4):
                        at = acp.tile([128, NL], BF16, tag=f"ach{b}")
                        nc.sync.dma_start(
                            out=at[:],
                            in_=a2a_a[block][b][2 * g:2 * g + 2, :, :].rearrange(
                                "c d n -> (c d) n"))
                        atiles.append(at)
                    for o in range(4):
                        ps = psC.tile([128, NL], F32, tag="cps")
                        for g in range(4):
                            nc.tensor.matmul(ps[:],
                                             wm_sb[:, g, o * 128:(o + 1) * 128],
                                             atiles[g][:],
                                             start=(g == 0), stop=(g == 3))
                        nc.vector.tensor_scalar(am_sb[:, o, b, :], ps[:],
                                                bm_sb[:, o:o + 1], None, ALU.add)
                    for o in range(8):
                        ps = psC.tile([128, NL], F32, tag="cps")
                        for g in range(8):
                            rhs = (x_sb[:, g, b, :] if g < 4
                                   else am_sb[:, g - 4, b, :])
                            nc.tensor.matmul(ps[:],
                                             w1_sb[:, g, o * 128:(o + 1) * 128],
                                             rhs, start=(g == 0), stop=(g == 7))
                        nc.vector.tensor_copy(h1_sb[:, o, b, :], ps[:])
                        nc.vector.tensor_reduce(stats[:, 2 * o:2 * o + 1],
                                                h1_sb[:, o, b, :], AX.X, ALU.add)
                        # tensor_tensor_reduce crashes this runtime; ACT
                        # Square with accum_out computes the sum of squares.
                        scr = scp.tile([128, NL], F32, tag="sq")
                        nc.scalar.activation(
                            scr[:], h1_sb[:, o, b, :], AF.Square,
                            accum_out=stats[:, 2 * o + 1:2 * o + 2])
                    # stats exchange: AllGather + local 8-way sum (cheaper
                    # than AllReduce's 1.875x wall-clock premium).
                    nc.sync.dma_start(out=cc_st_in[block][b][:], in_=stats[:])
                    nc.gpsimd.collective_compute(
                        "AllGather", ALU.bypass, replica_groups=RG,
                        ins=[cc_st_in[block][b][:]], outs=[ag_st[block][b][:]])
                so_sb = sop.tile([128, 4, B, NL], F32, tag="so")
                # per-batch tail: IN-math/relu/W2/out (+ k/v proj and its
                # AllToAll), so the b=0 kv AllToAll overlaps b=1's W2.
                for b in range(B):
                    stg8 = stp.tile([128, NCORES, 16], F32, tag=f"stg8{b}")
                    nc.sync.dma_start(
                        out=stg8[:],
                        in_=ag_st[block][b][:].rearrange("c p s -> p c s"))
                    stg = stp.tile([128, 16], F32, tag=f"stg{b}")
                    nc.vector.tensor_add(stg[:], stg8[:, 0, :], stg8[:, 1, :])
                    for c in range(2, NCORES):
                        nc.vector.tensor_add(stg[:], stg[:], stg8[:, c, :])
                    mean = stp.tile([128, 8], F32, tag=f"mean{b}")
                    vtmp = stp.tile([128, 8], F32, tag=f"vtmp{b}")
                    rstd = stp.tile([128, 8], F32, tag=f"rstd{b}")
                    nbias = stp.tile([128, 8], F32, tag=f"nbias{b}")
                    s1v = stg[:].rearrange("p (c two) -> p c two", two=2)
                    nc.vector.tensor_scalar(mean[:], s1v[:, :, 0], 1.0 / N,
                                            None, ALU.mult)
                    nc.vector.tensor_scalar(vtmp[:], s1v[:, :, 1], 1.0 / N,
                                            None, ALU.mult)
                    nc.vector.tensor_mul(nbias[:], mean[:], mean[:])
                    nc.vector.tensor_sub(vtmp[:], vtmp[:], nbias[:])
                    nc.vector.tensor_scalar(vtmp[:], vtmp[:], EPS, None, ALU.add)
                    nc.scalar.sqrt(vtmp[:], vtmp[:])
                    nc.vector.reciprocal(rstd[:], vtmp[:])
                    nc.vector.tensor_mul(nbias[:], mean[:], rstd[:])
                    nc.vector.tensor_scalar(nbias[:], nbias[:], -1.0, None,
                                            ALU.mult)
                    for o in range(8):
                        nc.scalar.activation(hr_sb[:, o, b, :], h1_sb[:, o, b, :],
                                             AF.Relu, bias=nbias[:, o:o + 1],
                                             scale=rstd[:, o:o + 1])
                    for o in range(4):
                        ps = psC.tile([128, NL], F32, tag="cps")
                        for g in range(8):
                            nc.tensor.matmul(ps[:],
                                             w2_sb[:, g, o * 128:(o + 1) * 128],
                                             hr_sb[:, g, b, :],
                                             start=(g == 0), stop=(g == 7))
                        nc.vector.tensor_scalar(so_sb[:, o, b, :], ps[:],
                                                b2_sb[:, o:o + 1], None, ALU.add)
                        nc.vector.tensor_add(so_sb[:, o, b, :], so_sb[:, o, b, :],
                                             x_sb[:, o, b, :])
                        nc.sync.dma_start(
                            out=out_dram[b, o * 128:(o + 1) * 128, :],
                            in_=so_sb[:, o, b, :])
                    if send_kv:
                        emit_kv_send(b, so_sb)

        def emit_kv_send(b, so_sb):
            """Sender-side block-2 k/vT projections for one batch element,
            then that batch's merged AllToAll."""
            with tc.tile_pool(name=f"kvS{b}", bufs=1) as kvp, \
                 tc.tile_pool(name=f"psS2{b}", bufs=2, space="PSUM") as psS2:
                sob = kvp.tile([128, 4, NL], BF16, tag="sob")
                nc.vector.tensor_copy(sob[:], so_sb[:, :, b, :])
                k2_sb = kvp.tile([128, 4, NL], BF16, tag="k2")
                v2_sb = kvp.tile([128, 2, F], BF16, tag="v2t")
                for t in range(4):
                    ps = psS2.tile([128, NL], F32, tag="s2ps")
                    for g in range(4):
                        nc.tensor.matmul(
                            ps[:], wk2_sb[:, g, t * 128:(t + 1) * 128],
                            sob[:, g, :],
                            start=(g == 0), stop=(g == 3))
                    nc.vector.tensor_copy(k2_sb[:, t, :], ps[:])
                for mt in range(2):
                    ps = psS2.tile([128, F], F32, tag="s2ps")
                    for g in range(4):
                        nc.tensor.matmul(
                            ps[:], sob[:, g, mt * 128:(mt + 1) * 128],
                            wv2_sb[:, g, :],
                            start=(g == 0), stop=(g == 3))
                    nc.vector.tensor_copy(v2_sb[:, mt, :], ps[:])
                for h in range(NCORES):
                    t, r = h // 2, (h % 2) * D
                    nc.sync.dma_start(out=cc_kv_in[b][h, 0],
                                      in_=k2_sb[r:r + D, t, :])
                    nc.sync.dma_start(
                        out=cc_kv_in[b][h, 1].rearrange(
                            "d n -> (d n)").rearrange(
                            "(mt p d) -> p mt d", mt=2, p=128),
                        in_=v2_sb[:, :, h * D:(h + 1) * D])
                nc.gpsimd.collective_compute(
                    "AllToAll", ALU.bypass, replica_groups=RG,
                    ins=[cc_kv_in[b][:]], outs=[a2a_kv[b][:]])

        def emit_kv_assembly_b(b, k_sb, vt_sb):
            nc.sync.dma_start(
                out=k_sb[:, b, :].rearrange("d (c n) -> d c n", c=NCORES),
                in_=a2a_kv[b][:, 0, :, :].rearrange("c d n -> d c n"))
            for c in range(NCORES):
                nc.sync.dma_start(
                    out=vt_sb[:, b, 2 * c:2 * c + 2, 0:D],
                    in_=a2a_kv[b][c, 1].rearrange(
                        "d n -> (d n)").rearrange(
                        "(mt p d) -> p mt d", mt=2, p=128))

        for rp in range(reps):
            with tc.tile_pool(name=f"qk0_{rp}", bufs=1) as qkp:
                q_sb = qkp.tile([D, B, N], BF16, tag="q")
                q2_sb = qkp.tile([D, B, N], BF16, tag="q2")
                k_sb = qkp.tile([D, B, N], BF16, tag="k")
                vt_sb = qkp.tile([128, B, 16, D + 1], BF16, tag="vt")
                araw = qkp.tile([D + 1, B, N], F32, tag="araw")
                a_sb = qkp.tile([D, B, N], BF16, tag="a")
                nc.vector.memset(vt_sb[:, :, :, D], 1.0)
                emit_kvq_proj_block0(k_sb, vt_sb, q2_sb)
                if rp == 0:
                    emit_heavy_weight_loads()
                emit_q_proj(src, q_sb, f"q0_{rp}")
                # ---- block 0 attention, batch-pipelined ----
                with tc.tile_pool(name=f"pB0_{rp}", bufs=2) as pp, \
                     tc.tile_pool(name=f"dB0_{rp}", bufs=2) as dnp, \
                     tc.tile_pool(name=f"psS0_{rp}", bufs=2, space="PSUM") as psS, \
                     tc.tile_pool(name=f"psBC0_{rp}", bufs=2, space="PSUM") as psBC, \
                     tc.tile_pool(name=f"psPV0_{rp}", bufs=2, space="PSUM") as psPV:
                    for b in range(B):
                        emit_attention_b(0, b, q_sb, k_sb, vt_sb, araw, a_sb,
                                         pp, dnp, psS, psBC, psPV)
                # ---- block 0 MLP (+ sender-side block-1 k/v) ----
                emit_mlp(0, x1c, src_out_c, send_kv=True)
                # ---- block 1 attention, batch-pipelined with kv arrival ----
                k1_sb = qkp.tile([D, B, N], BF16, tag="k1")
                vt1_sb = qkp.tile([128, B, 16, D + 1], BF16, tag="vt1")
                araw1 = qkp.tile([D + 1, B, N], F32, tag="araw1")
                a1_sb = qkp.tile([D, B, N], BF16, tag="a1")
                nc.vector.memset(vt1_sb[:, :, :, D], 1.0)
                with tc.tile_pool(name=f"pB1_{rp}", bufs=2) as pp, \
                     tc.tile_pool(name=f"dB1_{rp}", bufs=2) as dnp, \
                     tc.tile_pool(name=f"psS1_{rp}", bufs=2, space="PSUM") as psS, \
                     tc.tile_pool(name=f"psBC1_{rp}", bufs=2, space="PSUM") as psBC, \
                     tc.tile_pool(name=f"psPV1_{rp}", bufs=2, space="PSUM") as psPV:
                    for b in range(B):
                        emit_kv_assembly_b(b, k1_sb, vt1_sb)
                        emit_attention_b(1, b, q2_sb, k1_sb, vt1_sb, araw1,
                                         a1_sb, pp, dnp, psS, psBC, psPV)
            emit_mlp(1, x2c, tgt_out_c, send_kv=False)

    nc.finalize()
    return nc


def _prep_inputs(src, tgt, Wq, bq, Wk, bk, Wv, bv, Wm, bm, W1, b1, W2, b2):
    """Host-side slicing/permutation into the per-core in_maps."""
    def b16(x):
        return np.asarray(x, np.float32).astype(NPBF16)

    src16 = b16(src)
    tgt16 = b16(tgt)
    perm = np.arange(F).reshape(D, H).T.reshape(F)  # f' = h*64+d -> f = d*8+h
    wm_perm = b16(np.asarray(Wm).reshape(F, D, H).transpose(2, 1, 0).reshape(F, F))
    w1t = b16(np.asarray(W1).T)
    w2t = b16(np.asarray(W2).T)
    wk2t = b16(np.asarray(Wk)[perm, :].T)
    wv2t = b16(np.asarray(Wv)[perm, :].T)
    bm_c = np.ascontiguousarray(np.asarray(bm).reshape(F, 1), np.float32)
    b2_c = np.ascontiguousarray(np.asarray(b2).reshape(F, 1), np.float32)
    in_maps = []
    for h in range(NCORES):
        f_list = np.arange(D) * H + h
        sl = slice(h * NL, (h + 1) * NL)
        in_maps.append({
            "src": src16,
            "tgt": tgt16,
            "x1c": np.ascontiguousarray(src16[:, :, sl]),
            "x2c": np.ascontiguousarray(tgt16[:, :, sl]),
            "wq": b16(np.asarray(Wq)[f_list, :].T),
            "wk": b16(np.asarray(Wk)[f_list, :].T),
            "wv": b16(np.asarray(Wv)[f_list, :].T),
            "bq": np.ascontiguousarray(
                np.asarray(bq)[f_list].reshape(D, 1), np.float32),
            "bv": np.ascontiguousarray(
                np.asarray(bv)[f_list].reshape(D, 1), np.float32),
            "wmT": wm_perm,
            "bm": bm_c,
            "w1T": w1t,
            "w2T": w2t,
            "b2": b2_c,
            "wk2T": wk2t,
            "wv2T": wv2t,
        })
    return in_maps


def kernel(**inputs):
    if "nc" not in _CACHE:
        _CACHE["nc"] = build_nc()
    nc = _CACHE["nc"]
    in_maps = _prep_inputs(**inputs)
    res = run_bass_kernel_spmd(nc, in_maps, list(range(NCORES)))
    src_out = np.concatenate(
        [res.results[c]["src_out_c"] for c in range(NCORES)], axis=2)
    tgt_out = np.concatenate(
        [res.results[c]["tgt_out_c"] for c in range(NCORES)], axis=2)
    return (src_out, tgt_out)


# revision 13
# speedup vs baseline: 447.0714x; 1.0038x over previous
"""Two-way cross-attention block (SuperGlue-style) on 8 trn2 NeuronCores.

Sharding: attention is sharded by head (8 heads -> 8 cores); the MLP /
conv1x1(Wm,W1,W2) + instance-norm part is sharded by sequence position
(2048 -> 8 chunks of 256).  All cross-core collectives are split per
batch element and pipelined against the other batch's compute: the
attention-out AllToAll for b=0 flies while b=1's scores/softmax run, and
block-2's merged k/v AllToAll for b=1 is hidden under block-2 attention
on b=0.  Instance-norm sufficient stats use AllGather + a local 8-way
sum (cheaper than AllReduce in wall-clock, and only 16 KB/rank).

All matmul operands are bf16 (PE runs 4x faster than fp32; PSUM
accumulation stays fp32).  Exact-math rewrites vs the reference:
  * v-projection bias applied after softmax normalization (rows sum to 1).
  * k-projection bias dropped entirely: it shifts every score in a softmax
    row by the same q.bk constant, which cancels in the softmax.
  * W1's conv bias cancels inside the affine-free InstanceNorm -> dropped.
  * softmax without max-subtraction (scores are small, safe in fp32).
"""

import sys

if "/opt/trn_rl_repo" not in sys.path:
    sys.path.insert(0, "/opt/trn_rl_repo")

import numpy as np

import concourse.bass as bass
import concourse.mybir as mybir
import concourse.tile as tile
from concourse import bacc
from concourse.bass_utils import run_bass_kernel_spmd

F32 = mybir.dt.float32
BF16 = mybir.dt.bfloat16
NPBF16 = mybir.dt.np(mybir.dt.bfloat16)
AF = mybir.ActivationFunctionType
ALU = mybir.AluOpType
AX = mybir.AxisListType

B = 2        # batch
F = 512      # feature dim
H = 8        # heads
D = 64       # head depth
N = 2048     # sequence length
NL = N // 8  # per-core position chunk (256)
NCORES = 8
EPS = 1e-5
RG = [list(range(NCORES))]

_CACHE = {}


def build_nc(reps=1):
    nc = bacc.Bacc("TRN2", target_bir_lowering=False, num_devices=NCORES)

    # ---------------- external I/O (per core) ----------------
    src = nc.dram_tensor("src", [B, F, N], BF16, kind="ExternalInput")
    tgt = nc.dram_tensor("tgt", [B, F, N], BF16, kind="ExternalInput")
    x1c = nc.dram_tensor("x1c", [B, F, NL], BF16, kind="ExternalInput")
    x2c = nc.dram_tensor("x2c", [B, F, NL], BF16, kind="ExternalInput")
    wq = nc.dram_tensor("wq", [F, D], BF16, kind="ExternalInput")
    wk = nc.dram_tensor("wk", [F, D], BF16, kind="ExternalInput")
    wv = nc.dram_tensor("wv", [F, D], BF16, kind="ExternalInput")
    bq = nc.dram_tensor("bq", [D, 1], F32, kind="ExternalInput")
    bv = nc.dram_tensor("bv", [D, 1], F32, kind="ExternalInput")
    wmT = nc.dram_tensor("wmT", [F, F], BF16, kind="ExternalInput")
    bm = nc.dram_tensor("bm", [F, 1], F32, kind="ExternalInput")
    w1T = nc.dram_tensor("w1T", [2 * F, 2 * F], BF16, kind="ExternalInput")
    w2T = nc.dram_tensor("w2T", [2 * F, F], BF16, kind="ExternalInput")
    b2 = nc.dram_tensor("b2", [F, 1], F32, kind="ExternalInput")
    # block-2 sender-side projection weights (output channels head-major)
    wk2T = nc.dram_tensor("wk2T", [F, F], BF16, kind="ExternalInput")
    wv2T = nc.dram_tensor("wv2T", [F, F], BF16, kind="ExternalInput")

    src_out_c = nc.dram_tensor("src_out_c", [B, F, NL], F32, kind="ExternalOutput")
    tgt_out_c = nc.dram_tensor("tgt_out_c", [B, F, NL], F32, kind="ExternalOutput")

    # ------------- internal DRAM (per-batch-split collectives) -------------
    cc_a_in = [[nc.dram_tensor(f"cc_a_in{i}_{b}", [NCORES, D, NL], BF16)
                for b in range(B)] for i in range(2)]
    a2a_a = [[nc.dram_tensor(f"a2a_a{i}_{b}", [NCORES, D, NL], BF16)
              for b in range(B)] for i in range(2)]
    cc_st_in = [[nc.dram_tensor(f"cc_st_in{i}_{b}", [128, 16], F32)
                 for b in range(B)] for i in range(2)]
    ag_st = [[nc.dram_tensor(f"ag_st{i}_{b}", [NCORES, 128, 16], F32)
              for b in range(B)] for i in range(2)]
    # merged k/v AllToAll for block 2: slot h = [k rows of head h | vT cols]
    cc_kv_in = [nc.dram_tensor(f"cc_kv_in{b}", [NCORES, 2, D, NL], BF16)
                for b in range(B)]
    a2a_kv = [nc.dram_tensor(f"a2a_kv{b}", [NCORES, 2, D, NL], BF16)
              for b in range(B)]

    with tile.TileContext(nc) as tc, bass.ExitStack() as ctx:
        # ---------- persistent tiles ----------
        wp = ctx.enter_context(tc.tile_pool(name="weights", bufs=1))
        wq_sb = wp.tile([128, 4, D], BF16, tag="wq")
        wk_sb = wp.tile([128, 4, D], BF16, tag="wk")
        wv_sb = wp.tile([128, 4, D], BF16, tag="wv")
        bq_sb = wp.tile([D, 1], F32, tag="bq")
        bv_sb = wp.tile([D, 1], F32, tag="bv")
        wm_sb = wp.tile([128, 4, F], BF16, tag="wm")
        bm_sb = wp.tile([128, 4], F32, tag="bm")
        w1_sb = wp.tile([128, 8, 2 * F], BF16, tag="w1")
        w2_sb = wp.tile([128, 8, F], BF16, tag="w2")
        b2_sb = wp.tile([128, 4], F32, tag="b2")
        wk2_sb = wp.tile([128, 4, F], BF16, tag="wk2")
        wv2_sb = wp.tile([128, 4, F], BF16, tag="wv2")
        ones1 = wp.tile([1, D], BF16, tag="ones1")
        nc.vector.memset(ones1[:], 1.0)

        for t, d_ in ((wq_sb, wq), (wk_sb, wk), (wv_sb, wv)):
            nc.sync.dma_start(out=t[:], in_=d_[:].rearrange("(t p) d -> p t d", p=128))
        nc.sync.dma_start(out=bq_sb[:], in_=bq[:])
        nc.sync.dma_start(out=bv_sb[:], in_=bv[:])

        def emit_heavy_weight_loads():
            # emitted after the first src/tgt stream DMAs: these 2.2 MB of
            # MLP weights aren't needed until ~100us in, and issuing them
            # first would stall the projection matmuls behind them.
            for t, d_ in ((wm_sb, wmT), (w1_sb, w1T), (w2_sb, w2T),
                          (wk2_sb, wk2T), (wv2_sb, wv2T)):
                nc.sync.dma_start(out=t[:],
                                  in_=d_[:].rearrange("(t p) o -> p t o", p=128))
            for t, d_ in ((bm_sb, bm), (b2_sb, b2)):
                nc.sync.dma_start(
                    out=t[:], in_=d_[:].rearrange("(t p) one -> p (t one)", p=128))

        def emit_q_proj(qsrc_dram, q_sb, name):
            """q_sb[d, b, n] = (Wq_h @ qsrc + bq_h), streamed over f-tiles."""
            with tc.tile_pool(name=f"tb{name}", bufs=5) as tbp, \
                 tc.tile_pool(name=f"psQ{name}", bufs=4, space="PSUM") as psQ:
                for b in range(B):
                    qtiles = []
                    for kf in range(4):
                        t = tbp.tile([128, N], BF16, tag="kv")
                        nc.sync.dma_start(out=t[:],
                                          in_=qsrc_dram[b, kf * 128:(kf + 1) * 128, :])
                        qtiles.append(t)
                    for nt in range(4):
                        ps = psQ.tile([D, 512], F32, tag="qps")
                        for kf in range(4):
                            nc.tensor.matmul(ps[:], wq_sb[:, kf, :],
                                             qtiles[kf][:, nt * 512:(nt + 1) * 512],
                                             start=(kf == 0), stop=(kf == 3))
                        nc.vector.tensor_scalar(
                            q_sb[:, b, nt * 512:(nt + 1) * 512],
                            ps[:], bq_sb[:], None, ALU.add)

        def emit_kvq_proj_block0(k_sb, vt_sb, q2_sb):
            """Block-0 k and vT plus block-1 q projections, one tgt stream."""
            with tc.tile_pool(name="tbKV0", bufs=5) as tbp, \
                 tc.tile_pool(name="psK0", bufs=2, space="PSUM") as psA, \
                 tc.tile_pool(name="psQ2", bufs=2, space="PSUM") as psQ2, \
                 tc.tile_pool(name="psV0", bufs=2, space="PSUM") as psVT:
                for b in range(B):
                    tiles = []
                    for kf in range(4):
                        t = tbp.tile([128, N], BF16, tag="kv")
                        nc.sync.dma_start(out=t[:],
                                          in_=tgt[b, kf * 128:(kf + 1) * 128, :])
                        tiles.append(t)
                    for nt in range(4):
                        ps = psA.tile([D, 512], F32, tag="kps")
                        for kf in range(4):
                            nc.tensor.matmul(ps[:], wk_sb[:, kf, :],
                                             tiles[kf][:, nt * 512:(nt + 1) * 512],
                                             start=(kf == 0), stop=(kf == 3))
                        # PSUM drain on ACT: it is idle during projections,
                        # while DVE gates the q/q2 bias-add drains.
                        nc.scalar.activation(
                            k_sb[:, b, nt * 512:(nt + 1) * 512], ps[:], AF.Copy)
                        ps2 = psQ2.tile([D, 512], F32, tag="q2ps")
                        for kf in range(4):
                            nc.tensor.matmul(ps2[:], wq_sb[:, kf, :],
                                             tiles[kf][:, nt * 512:(nt + 1) * 512],
                                             start=(kf == 0), stop=(kf == 3))
                        nc.vector.tensor_scalar(
                            q2_sb[:, b, nt * 512:(nt + 1) * 512],
                            ps2[:], bq_sb[:], None, ALU.add)
                    for mg in range(4):
                        ps = psVT.tile([128, 4, D], F32, tag="vtps")
                        for j in range(4):
                            mi = 4 * mg + j
                            for kf in range(4):
                                nc.tensor.matmul(
                                    ps[:, j, :],
                                    tiles[kf][:, mi * 128:(mi + 1) * 128],
                                    wv_sb[:, kf, :],
                                    start=(kf == 0), stop=(kf == 3))
                        nc.scalar.activation(
                            vt_sb[:, b, 4 * mg:4 * mg + 4, 0:D], ps[:], AF.Copy)

        def emit_attention_b(block, b, q_sb, k_sb, vt_sb, araw, a_sb, pp, dnp,
                             psS, psBC, psPV):
            """One batch element of attention + its per-batch AllToAll."""
            for nb in range(4):
                nsl = slice(nb * 512, (nb + 1) * 512)
                p_sb = pp.tile([128, 16, 512], BF16, tag="p")
                for g in range(8):
                    ps = psS.tile([128, 2, 512], F32, tag="sps")
                    for j in range(2):
                        mi = 2 * g + j
                        nc.tensor.matmul(
                            ps[:, j, :],
                            k_sb[:, b, mi * 128:(mi + 1) * 128],
                            q_sb[:, b, nsl], start=True, stop=True)
                    nc.scalar.activation(p_sb[:, 2 * g:2 * g + 2, :], ps[:],
                                         AF.Exp, scale=float(1.0 / np.sqrt(D)))
                pv = psPV.tile([D + 1, 512], F32, tag="pvps")
                for mi in range(16):
                    nc.tensor.matmul(pv[:], vt_sb[:, b, mi, :],
                                     p_sb[:, mi, :],
                                     start=(mi == 0), stop=(mi == 15))
                nc.vector.tensor_copy(araw[:, b, nsl], pv[:])
            # normalize: a = araw[0:64]/araw[64] + bv; den row is bounced
            # to partition 0 by a small SBUF->SBUF DMA, the broadcast is
            # a K=1 ones-matmul on PE (gpsimd broadcast is ~11us/op).
            rden = dnp.tile([1, N], F32, tag="rden")
            rdenb = dnp.tile([1, N], BF16, tag="rdenb")
            nc.sync.dma_start(out=rden[:], in_=araw[D:D + 1, b, :])
            nc.vector.reciprocal(rden[:], rden[:])
            nc.vector.tensor_copy(rdenb[:], rden[:])
            for nb in range(4):
                nsl = slice(nb * 512, (nb + 1) * 512)
                bc = psBC.tile([D, 512], F32, tag="bc")
                nc.tensor.matmul(bc[:], ones1[:], rdenb[:, nsl],
                                 start=True, stop=True)
                nc.vector.tensor_mul(a_sb[:, b, nsl], araw[0:D, b, nsl],
                                     bc[:])
            nc.vector.tensor_scalar(a_sb[:, b, :], a_sb[:, b, :], bv_sb[:],
                                    None, ALU.add)
            nc.sync.dma_start(
                out=cc_a_in[block][b][:].rearrange("c d n -> d c n"),
                in_=a_sb[:, b, :].rearrange("d (c n) -> d c n", c=NCORES))
            nc.gpsimd.collective_compute(
                "AllToAll", ALU.bypass, replica_groups=RG,
                ins=[cc_a_in[block][b][:]], outs=[a2a_a[block][b][:]])

        def emit_mlp(block, x_chunk_dram, out_dram, send_kv):
            with tc.tile_pool(name=f"xC{block}", bufs=1) as xcp, \
                 tc.tile_pool(name=f"aC{block}", bufs=8) as acp, \
                 tc.tile_pool(name=f"amC{block}", bufs=1) as amp, \
                 tc.tile_pool(name=f"hC{block}", bufs=1) as hp, \
                 tc.tile_pool(name=f"stC{block}", bufs=1) as stp, \
                 tc.tile_pool(name=f"scrC{block}", bufs=2) as scp, \
                 tc.tile_pool(name=f"soC{block}", bufs=1) as sop, \
                 tc.tile_pool(name=f"psC{block}", bufs=6, space="PSUM") as psC:
                x_sb = xcp.tile([128, 4, B, NL], BF16, tag="x")
                for b in range(B):
                    nc.sync.dma_start(
                        out=x_sb[:, :, b, :],
                        in_=x_chunk_dram[b].rearrange("(t p) n -> p t n", p=128))
                am_sb = amp.tile([128, 4, B, NL], BF16, tag="am")
                h1_sb = hp.tile([128, 8, B, NL], BF16, tag="h1")
                hr_sb = hp.tile([128, 8, B, NL], BF16, tag="hr")
                # per-batch front: Wm + W1 + stats + that batch's stats
                # AllGather, so b=0's AG flies during b=1's Wm/W1 compute.
                for b in range(B):
                    stats = stp.tile([128, 16], F32, tag=f"st{b}")
                    atiles = []
                    for g in range(4):
                        at = acp.tile([128, NL], BF16, tag=f"ach{b}")
                        nc.sync.dma_start(
                            out=at[:],
                            in_=a2a_a[block][b][2 * g:2 * g + 2, :, :].rearrange(
                                "c d n -> (c d) n"))
                        atiles.append(at)
                    for o in range(4):
                        ps = psC.tile([128, NL], F32, tag="cps")
                        for g in range(4):
                            nc.tensor.matmul(ps[:],
                                             wm_sb[:, g, o * 128:(o + 1) * 128],
                                             atiles[g][:],
                                             start=(g == 0), stop=(g == 3))
                        nc.vector.tensor_scalar(am_sb[:, o, b, :], ps[:],
                                                bm_sb[:, o:o + 1], None, ALU.add)
                    for o in range(8):
                        ps = psC.tile([128, NL], F32, tag="cps")
                        for g in range(8):
                            rhs = (x_sb[:, g, b, :] if g < 4
                                   else am_sb[:, g - 4, b, :])
                            nc.tensor.matmul(ps[:],
                                             w1_sb[:, g, o * 128:(o + 1) * 128],
                                             rhs, start=(g == 0), stop=(g == 7))
                        nc.vector.tensor_copy(h1_sb[:, o, b, :], ps[:])
                        nc.vector.tensor_reduce(stats[:, 2 * o:2 * o + 1],
                                                h1_sb[:, o, b, :], AX.X, ALU.add)
                        # tensor_tensor_reduce crashes this runtime; ACT
                        # Square with accum_out computes the sum of squares.
                        scr = scp.tile([128, NL], F32, tag="sq")
                        nc.scalar.activation(
                            scr[:], h1_sb[:, o, b, :], AF.Square,
                            accum_out=stats[:, 2 * o + 1:2 * o + 2])
                    # stats exchange: AllGather + local 8-way sum (cheaper
                    # than AllReduce's 1.875x wall-clock premium).
                    nc.sync.dma_start(out=cc_st_in[block][b][:], in_=stats[:])
                    nc.gpsimd.collective_compute(
                        "AllGather", ALU.bypass, replica_groups=RG,
                        ins=[cc_st_in[block][b][:]], outs=[ag_st[block][b][:]])
                so_sb = sop.tile([128, 4, B, NL], F32, tag="so")
                # per-batch tail: IN-math/relu/W2/out (+ k/v proj and its
                # AllToAll), so the b=0 kv AllToAll overlaps b=1's W2.
                for b in range(B):
                    stg8 = stp.tile([128, NCORES, 16], F32, tag=f"stg8{b}")
                    nc.sync.dma_start(
                        out=stg8[:],
                        in_=ag_st[block][b][:].rearrange("c p s -> p c s"))
                    stg = stp.tile([128, 16], F32, tag=f"stg{b}")
                    nc.vector.tensor_add(stg[:], stg8[:, 0, :], stg8[:, 1, :])
                    for c in range(2, NCORES):
                        nc.vector.tensor_add(stg[:], stg[:], stg8[:, c, :])
                    mean = stp.tile([128, 8], F32, tag=f"mean{b}")
                    vtmp = stp.tile([128, 8], F32, tag=f"vtmp{b}")
                    rstd = stp.tile([128, 8], F32, tag=f"rstd{b}")
                    nbias = stp.tile([128, 8], F32, tag=f"nbias{b}")
                    s1v = stg[:].rearrange("p (c two) -> p c two", two=2)
                    nc.vector.tensor_scalar(mean[:], s1v[:, :, 0], 1.0 / N,
                                            None, ALU.mult)
                    nc.vector.tensor_scalar(vtmp[:], s1v[:, :, 1], 1.0 / N,
                                            None, ALU.mult)
                    nc.vector.tensor_mul(nbias[:], mean[:], mean[:])
                    nc.vector.tensor_sub(vtmp[:], vtmp[:], nbias[:])
                    nc.vector.tensor_scalar(vtmp[:], vtmp[:], EPS, None, ALU.add)
                    nc.scalar.sqrt(vtmp[:], vtmp[:])
                    nc.vector.reciprocal(rstd[:], vtmp[:])
                    nc.vector.tensor_mul(nbias[:], mean[:], rstd[:])
                    nc.vector.tensor_scalar(nbias[:], nbias[:], -1.0, None,
                                            ALU.mult)
                    for o in range(8):
                        nc.scalar.activation(hr_sb[:, o, b, :], h1_sb[:, o, b, :],
                                             AF.Relu, bias=nbias[:, o:o + 1],
                                             scale=rstd[:, o:o + 1])
                    for o in range(4):
                        ps = psC.tile([128, NL], F32, tag="cps")
                        for g in range(8):
                            nc.tensor.matmul(ps[:],
                                             w2_sb[:, g, o * 128:(o + 1) * 128],
                                             hr_sb[:, g, b, :],
                                             start=(g == 0), stop=(g == 7))
                        nc.vector.tensor_scalar(so_sb[:, o, b, :], ps[:],
                                                b2_sb[:, o:o + 1], None, ALU.add)
                        nc.vector.tensor_add(so_sb[:, o, b, :], so_sb[:, o, b, :],
                                             x_sb[:, o, b, :])
                        nc.sync.dma_start(
                            out=out_dram[b, o * 128:(o + 1) * 128, :],
                            in_=so_sb[:, o, b, :])
                    if send_kv:
                        emit_kv_send(b, so_sb)

        def emit_kv_send(b, so_sb):
            """Sender-side block-2 k/vT projections for one batch element,
            then that batch's merged AllToAll."""
            with tc.tile_pool(name=f"kvS{b}", bufs=1) as kvp, \
                 tc.tile_pool(name=f"psS2{b}", bufs=2, space="PSUM") as psS2:
                sob = kvp.tile([128, 4, NL], BF16, tag="sob")
                nc.vector.tensor_copy(sob[:], so_sb[:, :, b, :])
                k2_sb = kvp.tile([128, 4, NL], BF16, tag="k2")
                v2_sb = kvp.tile([128, 2, F], BF16, tag="v2t")
                for t in range(4):
                    ps = psS2.tile([128, NL], F32, tag="s2ps")
                    for g in range(4):
                        nc.tensor.matmul(
                            ps[:], wk2_sb[:, g, t * 128:(t + 1) * 128],
                            sob[:, g, :],
                            start=(g == 0), stop=(g == 3))
                    nc.vector.tensor_copy(k2_sb[:, t, :], ps[:])
                for mt in range(2):
                    ps = psS2.tile([128, F], F32, tag="s2ps")
                    for g in range(4):
                        nc.tensor.matmul(
                            ps[:], sob[:, g, mt * 128:(mt + 1) * 128],
                            wv2_sb[:, g, :],
                            start=(g == 0), stop=(g == 3))
                    nc.vector.tensor_copy(v2_sb[:, mt, :], ps[:])
                for h in range(NCORES):
                    t, r = h // 2, (h % 2) * D
                    nc.sync.dma_start(out=cc_kv_in[b][h, 0],
                                      in_=k2_sb[r:r + D, t, :])
                    nc.sync.dma_start(
                        out=cc_kv_in[b][h, 1].rearrange(
                            "d n -> (d n)").rearrange(
                            "(mt p d) -> p mt d", mt=2, p=128),
                        in_=v2_sb[:, :, h * D:(h + 1) * D])
                nc.gpsimd.collective_compute(
                    "AllToAll", ALU.bypass, replica_groups=RG,
                    ins=[cc_kv_in[b][:]], outs=[a2a_kv[b][:]])

        def emit_kv_assembly_b(b, k_sb, vt_sb):
            nc.sync.dma_start(
                out=k_sb[:, b, :].rearrange("d (c n) -> d c n", c=NCORES),
                in_=a2a_kv[b][:, 0, :, :].rearrange("c d n -> d c n"))
            for c in range(NCORES):
                nc.sync.dma_start(
                    out=vt_sb[:, b, 2 * c:2 * c + 2, 0:D],
                    in_=a2a_kv[b][c, 1].rearrange(
                        "d n -> (d n)").rearrange(
                        "(mt p d) -> p mt d", mt=2, p=128))

        for rp in range(reps):
            with tc.tile_pool(name=f"qk0_{rp}", bufs=1) as qkp:
                q_sb = qkp.tile([D, B, N], BF16, tag="q")
                q2_sb = qkp.tile([D, B, N], BF16, tag="q2")
                k_sb = qkp.tile([D, B, N], BF16, tag="k")
                vt_sb = qkp.tile([128, B, 16, D + 1], BF16, tag="vt")
                araw = qkp.tile([D + 1, B, N], F32, tag="araw")
                a_sb = qkp.tile([D, B, N], BF16, tag="a")
                nc.vector.memset(vt_sb[:, :, :, D], 1.0)
                emit_kvq_proj_block0(k_sb, vt_sb, q2_sb)
                if rp == 0:
                    emit_heavy_weight_loads()
                emit_q_proj(src, q_sb, f"q0_{rp}")
                # ---- block 0 attention, batch-pipelined ----
                with tc.tile_pool(name=f"pB0_{rp}", bufs=2) as pp, \
                     tc.tile_pool(name=f"dB0_{rp}", bufs=2) as dnp, \
                     tc.tile_pool(name=f"psS0_{rp}", bufs=2, space="PSUM") as psS, \
                     tc.tile_pool(name=f"psBC0_{rp}", bufs=2, space="PSUM") as psBC, \
                     tc.tile_pool(name=f"psPV0_{rp}", bufs=2, space="PSUM") as psPV:
                    for b in range(B):
                        emit_attention_b(0, b, q_sb, k_sb, vt_sb, araw, a_sb,
                                         pp, dnp, psS, psBC, psPV)
                # ---- block 0 MLP (+ sender-side block-1 k/v) ----
                emit_mlp(0, x1c, src_out_c, send_kv=True)
                # ---- block 1 attention, batch-pipelined with kv arrival ----
                k1_sb = qkp.tile([D, B, N], BF16, tag="k1")
                vt1_sb = qkp.tile([128, B, 16, D + 1], BF16, tag="vt1")
                araw1 = qkp.tile([D + 1, B, N], F32, tag="araw1")
                a1_sb = qkp.tile([D, B, N], BF16, tag="a1")
                nc.vector.memset(vt1_sb[:, :, :, D], 1.0)
                with tc.tile_pool(name=f"pB1_{rp}", bufs=2) as pp, \
                     tc.tile_pool(name=f"dB1_{rp}", bufs=2) as dnp, \
                     tc.tile_pool(name=f"psS1_{rp}", bufs=2, space="PSUM") as psS, \
                     tc.tile_pool(name=f"psBC1_{rp}", bufs=2, space="PSUM") as psBC, \
                     tc.tile_pool(name=f"psPV1_{rp}", bufs=2, space="PSUM") as psPV:
                    for b in range(B):
                        emit_kv_assembly_b(b, k1_sb, vt1_sb)
                        emit_attention_b(1, b, q2_sb, k1_sb, vt1_sb, araw1,
                                         a1_sb, pp, dnp, psS, psBC, psPV)
            emit_mlp(1, x2c, tgt_out_c, send_kv=False)

    nc.finalize()
    return nc


def _prep_inputs(src, tgt, Wq, bq, Wk, bk, Wv, bv, Wm, bm, W1, b1, W2, b2):
    """Host-side slicing/permutation into the per-core in_maps."""
    def b16(x):
        return np.asarray(x, np.float32).astype(NPBF16)

    src16 = b16(src)
    tgt16 = b16(tgt)
    perm = np.arange(F).reshape(D, H).T.reshape(F)  # f' = h*64+d -> f = d*8+h
    wm_perm = b16(np.asarray(Wm).reshape(F, D, H).transpose(2, 1, 0).reshape(F, F))
    w1t = b16(np.asarray(W1).T)
    w2t = b16(np.asarray(W2).T)
    wk2t = b16(np.asarray(Wk)[perm, :].T)
    wv2t = b16(np.asarray(Wv)[perm, :].T)
    bm_c = np.ascontiguousarray(np.asarray(bm).reshape(F, 1), np.float32)
    b2_c = np.ascontiguousarray(np.asarray(b2).reshape(F, 1), np.float32)
    in_maps = []
    for h in range(NCORES):
        f_list = np.arange(D) * H + h
        sl = slice(h * NL, (h + 1) * NL)
        in_maps.append({
            "src": src16,
            "tgt": tgt16,
            "x1c": np.ascontiguousarray(src16[:, :, sl]),
            "x2c": np.ascontiguousarray(tgt16[:, :, sl]),
            "wq": b16(np.asarray(Wq)[f_list, :].T),
            "wk": b16(np.asarray(Wk)[f_list, :].T),
            "wv": b16(np.asarray(Wv)[f_list, :].T),
            "bq": np.ascontiguousarray(
                np.asarray(bq)[f_list].reshape(D, 1), np.float32),
            "bv": np.ascontiguousarray(
                np.asarray(bv)[f_list].reshape(D, 1), np.float32),
            "wmT": wm_perm,
            "bm": bm_c,
            "w1T": w1t,
            "w2T": w2t,
            "b2": b2_c,
            "wk2T": wk2t,
            "wv2T": wv2t,
        })
    return in_maps


def kernel(**inputs):
    if "nc" not in _CACHE:
        _CACHE["nc"] = build_nc()
    nc = _CACHE["nc"]
    in_maps = _prep_inputs(**inputs)
    res = run_bass_kernel_spmd(nc, in_maps, list(range(NCORES)))
    src_out = np.concatenate(
        [res.results[c]["src_out_c"] for c in range(NCORES)], axis=2)
    tgt_out = np.concatenate(
        [res.results[c]["tgt_out_c"] for c in range(NCORES)], axis=2)
    return (src_out, tgt_out)


# revision 16
# speedup vs baseline: 448.9895x; 1.0043x over previous
"""Two-way cross-attention block (SuperGlue-style) on 8 trn2 NeuronCores.

Sharding: attention is sharded by head (8 heads -> 8 cores); the MLP /
conv1x1(Wm,W1,W2) + instance-norm part is sharded by sequence position
(2048 -> 8 chunks of 256).  All cross-core collectives are split per
batch element and pipelined against the other batch's compute: the
attention-out AllToAll for b=0 flies while b=1's scores/softmax run, and
block-2's merged k/v AllToAll for b=1 is hidden under block-2 attention
on b=0.  Instance-norm sufficient stats use AllGather + a local 8-way
sum (cheaper than AllReduce in wall-clock, and only 16 KB/rank).

All matmul operands are bf16 (PE runs 4x faster than fp32; PSUM
accumulation stays fp32).  Exact-math rewrites vs the reference:
  * v-projection bias applied after softmax normalization (rows sum to 1).
  * k-projection bias dropped entirely: it shifts every score in a softmax
    row by the same q.bk constant, which cancels in the softmax.
  * W1's conv bias cancels inside the affine-free InstanceNorm -> dropped.
  * softmax without max-subtraction (scores are small, safe in fp32).
"""

import sys

if "/opt/trn_rl_repo" not in sys.path:
    sys.path.insert(0, "/opt/trn_rl_repo")

import numpy as np

import concourse.bass as bass
import concourse.mybir as mybir
import concourse.tile as tile
from concourse import bacc
from concourse.bass_utils import run_bass_kernel_spmd

F32 = mybir.dt.float32
BF16 = mybir.dt.bfloat16
NPBF16 = mybir.dt.np(mybir.dt.bfloat16)
AF = mybir.ActivationFunctionType
ALU = mybir.AluOpType
AX = mybir.AxisListType

B = 2        # batch
F = 512      # feature dim
H = 8        # heads
D = 64       # head depth
N = 2048     # sequence length
NL = N // 8  # per-core position chunk (256)
NCORES = 8
EPS = 1e-5
RG = [list(range(NCORES))]

_CACHE = {}


def build_nc(reps=1):
    nc = bacc.Bacc("TRN2", target_bir_lowering=False, num_devices=NCORES)

    # ---------------- external I/O (per core) ----------------
    src = nc.dram_tensor("src", [B, F, N], BF16, kind="ExternalInput")
    tgt = nc.dram_tensor("tgt", [B, F, N], BF16, kind="ExternalInput")
    x1c = nc.dram_tensor("x1c", [B, F, NL], BF16, kind="ExternalInput")
    x2c = nc.dram_tensor("x2c", [B, F, NL], BF16, kind="ExternalInput")
    wq = nc.dram_tensor("wq", [F, D], BF16, kind="ExternalInput")
    wk = nc.dram_tensor("wk", [F, D], BF16, kind="ExternalInput")
    wv = nc.dram_tensor("wv", [F, D], BF16, kind="ExternalInput")
    bq = nc.dram_tensor("bq", [D, 1], F32, kind="ExternalInput")
    bv = nc.dram_tensor("bv", [D, 1], F32, kind="ExternalInput")
    wmT = nc.dram_tensor("wmT", [F, F], BF16, kind="ExternalInput")
    bm = nc.dram_tensor("bm", [F, 1], F32, kind="ExternalInput")
    w1T = nc.dram_tensor("w1T", [2 * F, 2 * F], BF16, kind="ExternalInput")
    w2T = nc.dram_tensor("w2T", [2 * F, F], BF16, kind="ExternalInput")
    b2 = nc.dram_tensor("b2", [F, 1], F32, kind="ExternalInput")
    # block-2 sender-side projection weights (output channels head-major)
    wk2T = nc.dram_tensor("wk2T", [F, F], BF16, kind="ExternalInput")
    wv2T = nc.dram_tensor("wv2T", [F, F], BF16, kind="ExternalInput")

    src_out_c = nc.dram_tensor("src_out_c", [B, F, NL], F32, kind="ExternalOutput")
    tgt_out_c = nc.dram_tensor("tgt_out_c", [B, F, NL], F32, kind="ExternalOutput")

    # ------------- internal DRAM (per-batch-split collectives) -------------
    cc_a_in = [[nc.dram_tensor(f"cc_a_in{i}_{b}", [NCORES, D, NL], BF16)
                for b in range(B)] for i in range(2)]
    a2a_a = [[nc.dram_tensor(f"a2a_a{i}_{b}", [NCORES, D, NL], BF16)
              for b in range(B)] for i in range(2)]
    cc_st_in = [[nc.dram_tensor(f"cc_st_in{i}_{b}", [128, 16], F32)
                 for b in range(B)] for i in range(2)]
    ag_st = [[nc.dram_tensor(f"ag_st{i}_{b}", [NCORES, 128, 16], F32)
              for b in range(B)] for i in range(2)]
    # merged k/v AllToAll for block 2: slot h = [k rows of head h | vT cols]
    cc_kv_in = [nc.dram_tensor(f"cc_kv_in{b}", [NCORES, 2, D, NL], BF16)
                for b in range(B)]
    a2a_kv = [nc.dram_tensor(f"a2a_kv{b}", [NCORES, 2, D, NL], BF16)
              for b in range(B)]

    with tile.TileContext(nc) as tc, bass.ExitStack() as ctx:
        # ---------- persistent tiles ----------
        wp = ctx.enter_context(tc.tile_pool(name="weights", bufs=1))
        wq_sb = wp.tile([128, 4, D], BF16, tag="wq")
        wk_sb = wp.tile([128, 4, D], BF16, tag="wk")
        wv_sb = wp.tile([128, 4, D], BF16, tag="wv")
        bq_sb = wp.tile([D, 1], F32, tag="bq")
        bv_sb = wp.tile([D, 1], F32, tag="bv")
        wm_sb = wp.tile([128, 4, F], BF16, tag="wm")
        bm_sb = wp.tile([128, 4], F32, tag="bm")
        w1_sb = wp.tile([128, 8, 2 * F], BF16, tag="w1")
        w2_sb = wp.tile([128, 8, F], BF16, tag="w2")
        b2_sb = wp.tile([128, 4], F32, tag="b2")
        wk2_sb = wp.tile([128, 4, F], BF16, tag="wk2")
        wv2_sb = wp.tile([128, 4, F], BF16, tag="wv2")
        ones1 = wp.tile([1, D], BF16, tag="ones1")
        nc.vector.memset(ones1[:], 1.0)

        for t, d_ in ((wq_sb, wq), (wk_sb, wk), (wv_sb, wv)):
            nc.sync.dma_start(out=t[:], in_=d_[:].rearrange("(t p) d -> p t d", p=128))
        nc.sync.dma_start(out=bq_sb[:], in_=bq[:])
        nc.sync.dma_start(out=bv_sb[:], in_=bv[:])

        def emit_heavy_weight_loads():
            # emitted after the first src/tgt stream DMAs: these 2.2 MB of
            # MLP weights aren't needed until ~100us in, and issuing them
            # first would stall the projection matmuls behind them.
            for t, d_ in ((wm_sb, wmT), (w1_sb, w1T), (w2_sb, w2T),
                          (wk2_sb, wk2T), (wv2_sb, wv2T)):
                nc.sync.dma_start(out=t[:],
                                  in_=d_[:].rearrange("(t p) o -> p t o", p=128))
            for t, d_ in ((bm_sb, bm), (b2_sb, b2)):
                nc.sync.dma_start(
                    out=t[:], in_=d_[:].rearrange("(t p) one -> p (t one)", p=128))

        def emit_q_proj(qsrc_dram, q_sb, name):
            """q_sb[d, b, n] = (Wq_h @ qsrc + bq_h), streamed over f-tiles."""
            with tc.tile_pool(name=f"tb{name}", bufs=2) as tbp, \
                 tc.tile_pool(name=f"psQ{name}", bufs=4, space="PSUM") as psQ:
                for b in range(B):
                    t4 = tbp.tile([128, 4, N], BF16, tag="kv")
                    nc.sync.dma_start(
                        out=t4[:],
                        in_=qsrc_dram[b].rearrange("(t p) n -> p t n", p=128))
                    for nt in range(4):
                        ps = psQ.tile([D, 512], F32, tag="qps")
                        for kf in range(4):
                            nc.tensor.matmul(ps[:], wq_sb[:, kf, :],
                                             t4[:, kf, nt * 512:(nt + 1) * 512],
                                             start=(kf == 0), stop=(kf == 3))
                        nc.vector.tensor_scalar(
                            q_sb[:, b, nt * 512:(nt + 1) * 512],
                            ps[:], bq_sb[:], None, ALU.add)

        def emit_kvq_proj_block0(k_sb, vt_sb, q2_sb):
            """Block-0 k and vT plus block-1 q projections, one tgt stream."""
            with tc.tile_pool(name="tbKV0", bufs=2) as tbp, \
                 tc.tile_pool(name="psK0", bufs=2, space="PSUM") as psA, \
                 tc.tile_pool(name="psQ2", bufs=2, space="PSUM") as psQ2, \
                 tc.tile_pool(name="psV0", bufs=2, space="PSUM") as psVT:
                for b in range(B):
                    t4 = tbp.tile([128, 4, N], BF16, tag="kv")
                    nc.sync.dma_start(
                        out=t4[:],
                        in_=tgt[b].rearrange("(t p) n -> p t n", p=128))
                    for nt in range(4):
                        ps = psA.tile([D, 512], F32, tag="kps")
                        for kf in range(4):
                            nc.tensor.matmul(ps[:], wk_sb[:, kf, :],
                                             t4[:, kf, nt * 512:(nt + 1) * 512],
                                             start=(kf == 0), stop=(kf == 3))
                        # PSUM drain on ACT: it is idle during projections,
                        # while DVE gates the q/q2 bias-add drains.
                        nc.scalar.activation(
                            k_sb[:, b, nt * 512:(nt + 1) * 512], ps[:], AF.Copy)
                        ps2 = psQ2.tile([D, 512], F32, tag="q2ps")
                        for kf in range(4):
                            nc.tensor.matmul(ps2[:], wq_sb[:, kf, :],
                                             t4[:, kf, nt * 512:(nt + 1) * 512],
                                             start=(kf == 0), stop=(kf == 3))
                        nc.vector.tensor_scalar(
                            q2_sb[:, b, nt * 512:(nt + 1) * 512],
                            ps2[:], bq_sb[:], None, ALU.add)
                    for mg in range(4):
                        ps = psVT.tile([128, 4, D], F32, tag="vtps")
                        for j in range(4):
                            mi = 4 * mg + j
                            for kf in range(4):
                                nc.tensor.matmul(
                                    ps[:, j, :],
                                    t4[:, kf, mi * 128:(mi + 1) * 128],
                                    wv_sb[:, kf, :],
                                    start=(kf == 0), stop=(kf == 3))
                        nc.scalar.activation(
                            vt_sb[:, b, 4 * mg:4 * mg + 4, 0:D], ps[:], AF.Copy)

        def emit_attention_b(block, b, q_sb, k_sb, vt_sb, araw, a_sb, pp, dnp,
                             psS, psBC, psPV):
            """One batch element of attention + its per-batch AllToAll."""
            for nb in range(4):
                nsl = slice(nb * 512, (nb + 1) * 512)
                p_sb = pp.tile([128, 16, 512], BF16, tag="p")
                for g in range(8):
                    ps = psS.tile([128, 2, 512], F32, tag="sps")
                    for j in range(2):
                        mi = 2 * g + j
                        nc.tensor.matmul(
                            ps[:, j, :],
                            k_sb[:, b, mi * 128:(mi + 1) * 128],
                            q_sb[:, b, nsl], start=True, stop=True)
                    nc.scalar.activation(p_sb[:, 2 * g:2 * g + 2, :], ps[:],
                                         AF.Exp, scale=float(1.0 / np.sqrt(D)))
                pv = psPV.tile([D + 1, 512], F32, tag="pvps")
                for mi in range(16):
                    nc.tensor.matmul(pv[:], vt_sb[:, b, mi, :],
                                     p_sb[:, mi, :],
                                     start=(mi == 0), stop=(mi == 15))
                nc.vector.tensor_copy(araw[:, b, nsl], pv[:])
            # normalize: a = araw[0:64]/araw[64] + bv; den row is bounced
            # to partition 0 by a small SBUF->SBUF DMA, the broadcast is
            # a K=1 ones-matmul on PE (gpsimd broadcast is ~11us/op).
            rden = dnp.tile([1, N], F32, tag="rden")
            rdenb = dnp.tile([1, N], BF16, tag="rdenb")
            nc.sync.dma_start(out=rden[:], in_=araw[D:D + 1, b, :])
            nc.vector.reciprocal(rden[:], rden[:])
            nc.vector.tensor_copy(rdenb[:], rden[:])
            for nb in range(4):
                nsl = slice(nb * 512, (nb + 1) * 512)
                bc = psBC.tile([D, 512], F32, tag="bc")
                nc.tensor.matmul(bc[:], ones1[:], rdenb[:, nsl],
                                 start=True, stop=True)
                nc.vector.tensor_mul(a_sb[:, b, nsl], araw[0:D, b, nsl],
                                     bc[:])
            nc.vector.tensor_scalar(a_sb[:, b, :], a_sb[:, b, :], bv_sb[:],
                                    None, ALU.add)
            nc.sync.dma_start(
                out=cc_a_in[block][b][:].rearrange("c d n -> d c n"),
                in_=a_sb[:, b, :].rearrange("d (c n) -> d c n", c=NCORES))
            nc.gpsimd.collective_compute(
                "AllToAll", ALU.bypass, replica_groups=RG,
                ins=[cc_a_in[block][b][:]], outs=[a2a_a[block][b][:]])

        def emit_mlp(block, x_chunk_dram, out_dram, send_kv):
            with tc.tile_pool(name=f"xC{block}", bufs=1) as xcp, \
                 tc.tile_pool(name=f"aC{block}", bufs=8) as acp, \
                 tc.tile_pool(name=f"amC{block}", bufs=1) as amp, \
                 tc.tile_pool(name=f"hC{block}", bufs=1) as hp, \
                 tc.tile_pool(name=f"stC{block}", bufs=1) as stp, \
                 tc.tile_pool(name=f"scrC{block}", bufs=2) as scp, \
                 tc.tile_pool(name=f"soC{block}", bufs=1) as sop, \
                 tc.tile_pool(name=f"psC{block}", bufs=6, space="PSUM") as psC:
                x_sb = xcp.tile([128, 4, B, NL], BF16, tag="x")
                for b in range(B):
                    nc.sync.dma_start(
                        out=x_sb[:, :, b, :],
                        in_=x_chunk_dram[b].rearrange("(t p) n -> p t n", p=128))
                am_sb = amp.tile([128, 4, B, NL], BF16, tag="am")
                h1_sb = hp.tile([128, 8, B, NL], BF16, tag="h1")
                hr_sb = hp.tile([128, 8, B, NL], BF16, tag="hr")
                # per-batch front: Wm + W1 + stats + that batch's stats
                # AllGather, so b=0's AG flies during b=1's Wm/W1 compute.
                for b in range(B):
                    stats = stp.tile([128, 16], F32, tag=f"st{b}")
                    # one batched DMA for all 4 a-tiles: fewer SP slots and a
                    # single 900ns sem-propagation on the A2A -> Wm path.
                    at4 = acp.tile([128, 4, NL], BF16, tag=f"ach{b}")
                    nc.sync.dma_start(
                        out=at4[:],
                        in_=a2a_a[block][b][:].rearrange(
                            "(g cc) d n -> (cc d) g n", cc=2))
                    for o in range(4):
                        ps = psC.tile([128, NL], F32, tag="cps")
                        for g in range(4):
                            nc.tensor.matmul(ps[:],
                                             wm_sb[:, g, o * 128:(o + 1) * 128],
                                             at4[:, g, :],
                                             start=(g == 0), stop=(g == 3))
                        nc.vector.tensor_scalar(am_sb[:, o, b, :], ps[:],
                                                bm_sb[:, o:o + 1], None, ALU.add)
                    for o in range(8):
                        ps = psC.tile([128, NL], F32, tag="cps")
                        for g in range(8):
                            rhs = (x_sb[:, g, b, :] if g < 4
                                   else am_sb[:, g - 4, b, :])
                            nc.tensor.matmul(ps[:],
                                             w1_sb[:, g, o * 128:(o + 1) * 128],
                                             rhs, start=(g == 0), stop=(g == 7))
                        nc.vector.tensor_copy(h1_sb[:, o, b, :], ps[:])
                        nc.vector.tensor_reduce(stats[:, 2 * o:2 * o + 1],
                                                h1_sb[:, o, b, :], AX.X, ALU.add)
                        # tensor_tensor_reduce crashes this runtime; ACT
                        # Square with accum_out computes the sum of squares.
                        scr = scp.tile([128, NL], F32, tag="sq")
                        nc.scalar.activation(
                            scr[:], h1_sb[:, o, b, :], AF.Square,
                            accum_out=stats[:, 2 * o + 1:2 * o + 2])
                    # stats exchange: AllGather + local 8-way sum (cheaper
                    # than AllReduce's 1.875x wall-clock premium).
                    nc.sync.dma_start(out=cc_st_in[block][b][:], in_=stats[:])
                    nc.gpsimd.collective_compute(
                        "AllGather", ALU.bypass, replica_groups=RG,
                        ins=[cc_st_in[block][b][:]], outs=[ag_st[block][b][:]])
                so_sb = sop.tile([128, 4, B, NL], F32, tag="so")
                # per-batch tail: IN-math/relu/W2/out (+ k/v proj and its
                # AllToAll), so the b=0 kv AllToAll overlaps b=1's W2.
                for b in range(B):
                    stg8 = stp.tile([128, NCORES, 16], F32, tag=f"stg8{b}")
                    nc.sync.dma_start(
                        out=stg8[:],
                        in_=ag_st[block][b][:].rearrange("c p s -> p c s"))
                    stg = stp.tile([128, 16], F32, tag=f"stg{b}")
                    nc.vector.tensor_add(stg[:], stg8[:, 0, :], stg8[:, 1, :])
                    for c in range(2, NCORES):
                        nc.vector.tensor_add(stg[:], stg[:], stg8[:, c, :])
                    mean = stp.tile([128, 8], F32, tag=f"mean{b}")
                    vtmp = stp.tile([128, 8], F32, tag=f"vtmp{b}")
                    rstd = stp.tile([128, 8], F32, tag=f"rstd{b}")
                    nbias = stp.tile([128, 8], F32, tag=f"nbias{b}")
                    s1v = stg[:].rearrange("p (c two) -> p c two", two=2)
                    nc.vector.tensor_scalar(mean[:], s1v[:, :, 0], 1.0 / N,
                                            None, ALU.mult)
                    nc.vector.tensor_scalar(vtmp[:], s1v[:, :, 1], 1.0 / N,
                                            None, ALU.mult)
                    nc.vector.tensor_mul(nbias[:], mean[:], mean[:])
                    nc.vector.tensor_sub(vtmp[:], vtmp[:], nbias[:])
                    nc.vector.tensor_scalar(vtmp[:], vtmp[:], EPS, None, ALU.add)
                    nc.scalar.sqrt(vtmp[:], vtmp[:])
                    nc.vector.reciprocal(rstd[:], vtmp[:])
                    nc.vector.tensor_mul(nbias[:], mean[:], rstd[:])
                    nc.vector.tensor_scalar(nbias[:], nbias[:], -1.0, None,
                                            ALU.mult)
                    for o in range(8):
                        nc.scalar.activation(hr_sb[:, o, b, :], h1_sb[:, o, b, :],
                                             AF.Relu, bias=nbias[:, o:o + 1],
                                             scale=rstd[:, o:o + 1])
                    for o in range(4):
                        ps = psC.tile([128, NL], F32, tag="cps")
                        for g in range(8):
                            nc.tensor.matmul(ps[:],
                                             w2_sb[:, g, o * 128:(o + 1) * 128],
                                             hr_sb[:, g, b, :],
                                             start=(g == 0), stop=(g == 7))
                        nc.vector.tensor_scalar(so_sb[:, o, b, :], ps[:],
                                                b2_sb[:, o:o + 1], None, ALU.add)
                        nc.vector.tensor_add(so_sb[:, o, b, :], so_sb[:, o, b, :],
                                             x_sb[:, o, b, :])
                        nc.sync.dma_start(
                            out=out_dram[b, o * 128:(o + 1) * 128, :],
                            in_=so_sb[:, o, b, :])
                    if send_kv:
                        emit_kv_send(b, so_sb)

        def emit_kv_send(b, so_sb):
            """Sender-side block-2 k/vT projections for one batch element,
            then that batch's merged AllToAll."""
            with tc.tile_pool(name=f"kvS{b}", bufs=1) as kvp, \
                 tc.tile_pool(name=f"psS2{b}", bufs=2, space="PSUM") as psS2:
                sob = kvp.tile([128, 4, NL], BF16, tag="sob")
                nc.vector.tensor_copy(sob[:], so_sb[:, :, b, :])
                k2_sb = kvp.tile([128, 4, NL], BF16, tag="k2")
                v2_sb = kvp.tile([128, 2, F], BF16, tag="v2t")
                for t in range(4):
                    ps = psS2.tile([128, NL], F32, tag="s2ps")
                    for g in range(4):
                        nc.tensor.matmul(
                            ps[:], wk2_sb[:, g, t * 128:(t + 1) * 128],
                            sob[:, g, :],
                            start=(g == 0), stop=(g == 3))
                    nc.vector.tensor_copy(k2_sb[:, t, :], ps[:])
                for mt in range(2):
                    ps = psS2.tile([128, F], F32, tag="s2ps")
                    for g in range(4):
                        nc.tensor.matmul(
                            ps[:], sob[:, g, mt * 128:(mt + 1) * 128],
                            wv2_sb[:, g, :],
                            start=(g == 0), stop=(g == 3))
                    nc.vector.tensor_copy(v2_sb[:, mt, :], ps[:])
                for h in range(NCORES):
                    t, r = h // 2, (h % 2) * D
                    nc.sync.dma_start(out=cc_kv_in[b][h, 0],
                                      in_=k2_sb[r:r + D, t, :])
                    nc.sync.dma_start(
                        out=cc_kv_in[b][h, 1].rearrange(
                            "d n -> (d n)").rearrange(
                            "(mt p d) -> p mt d", mt=2, p=128),
                        in_=v2_sb[:, :, h * D:(h + 1) * D])
                nc.gpsimd.collective_compute(
                    "AllToAll", ALU.bypass, replica_groups=RG,
                    ins=[cc_kv_in[b][:]], outs=[a2a_kv[b][:]])

        def emit_kv_assembly_b(b, k_sb, vt_sb):
            nc.sync.dma_start(
                out=k_sb[:, b, :].rearrange("d (c n) -> d c n", c=NCORES),
                in_=a2a_kv[b][:, 0, :, :].rearrange("c d n -> d c n"))
            for c in range(NCORES):
                nc.sync.dma_start(
                    out=vt_sb[:, b, 2 * c:2 * c + 2, 0:D],
                    in_=a2a_kv[b][c, 1].rearrange(
                        "d n -> (d n)").rearrange(
                        "(mt p d) -> p mt d", mt=2, p=128))

        for rp in range(reps):
            with tc.tile_pool(name=f"qk0_{rp}", bufs=1) as qkp:
                q_sb = qkp.tile([D, B, N], BF16, tag="q")
                q2_sb = qkp.tile([D, B, N], BF16, tag="q2")
                k_sb = qkp.tile([D, B, N], BF16, tag="k")
                vt_sb = qkp.tile([128, B, 16, D + 1], BF16, tag="vt")
                araw = qkp.tile([D + 1, B, N], F32, tag="araw")
                a_sb = qkp.tile([D, B, N], BF16, tag="a")
                nc.vector.memset(vt_sb[:, :, :, D], 1.0)
                emit_kvq_proj_block0(k_sb, vt_sb, q2_sb)
                if rp == 0:
                    emit_heavy_weight_loads()
                emit_q_proj(src, q_sb, f"q0_{rp}")
                # ---- block 0 attention, batch-pipelined ----
                with tc.tile_pool(name=f"pB0_{rp}", bufs=2) as pp, \
                     tc.tile_pool(name=f"dB0_{rp}", bufs=2) as dnp, \
                     tc.tile_pool(name=f"psS0_{rp}", bufs=2, space="PSUM") as psS, \
                     tc.tile_pool(name=f"psBC0_{rp}", bufs=2, space="PSUM") as psBC, \
                     tc.tile_pool(name=f"psPV0_{rp}", bufs=2, space="PSUM") as psPV:
                    for b in range(B):
                        emit_attention_b(0, b, q_sb, k_sb, vt_sb, araw, a_sb,
                                         pp, dnp, psS, psBC, psPV)
                # ---- block 0 MLP (+ sender-side block-1 k/v) ----
                emit_mlp(0, x1c, src_out_c, send_kv=True)
                # ---- block 1 attention, batch-pipelined with kv arrival ----
                k1_sb = qkp.tile([D, B, N], BF16, tag="k1")
                vt1_sb = qkp.tile([128, B, 16, D + 1], BF16, tag="vt1")
                araw1 = qkp.tile([D + 1, B, N], F32, tag="araw1")
                a1_sb = qkp.tile([D, B, N], BF16, tag="a1")
                nc.vector.memset(vt1_sb[:, :, :, D], 1.0)
                with tc.tile_pool(name=f"pB1_{rp}", bufs=2) as pp, \
                     tc.tile_pool(name=f"dB1_{rp}", bufs=2) as dnp, \
                     tc.tile_pool(name=f"psS1_{rp}", bufs=2, space="PSUM") as psS, \
                     tc.tile_pool(name=f"psBC1_{rp}", bufs=2, space="PSUM") as psBC, \
                     tc.tile_pool(name=f"psPV1_{rp}", bufs=2, space="PSUM") as psPV:
                    for b in range(B):
                        emit_kv_assembly_b(b, k1_sb, vt1_sb)
                        emit_attention_b(1, b, q2_sb, k1_sb, vt1_sb, araw1,
                                         a1_sb, pp, dnp, psS, psBC, psPV)
            emit_mlp(1, x2c, tgt_out_c, send_kv=False)

    nc.finalize()
    return nc


def _prep_inputs(src, tgt, Wq, bq, Wk, bk, Wv, bv, Wm, bm, W1, b1, W2, b2):
    """Host-side slicing/permutation into the per-core in_maps."""
    def b16(x):
        return np.asarray(x, np.float32).astype(NPBF16)

    src16 = b16(src)
    tgt16 = b16(tgt)
    perm = np.arange(F).reshape(D, H).T.reshape(F)  # f' = h*64+d -> f = d*8+h
    wm_perm = b16(np.asarray(Wm).reshape(F, D, H).transpose(2, 1, 0).reshape(F, F))
    w1t = b16(np.asarray(W1).T)
    w2t = b16(np.asarray(W2).T)
    wk2t = b16(np.asarray(Wk)[perm, :].T)
    wv2t = b16(np.asarray(Wv)[perm, :].T)
    bm_c = np.ascontiguousarray(np.asarray(bm).reshape(F, 1), np.float32)
    b2_c = np.ascontiguousarray(np.asarray(b2).reshape(F, 1), np.float32)
    in_maps = []
    for h in range(NCORES):
        f_list = np.arange(D) * H + h
        sl = slice(h * NL, (h + 1) * NL)
        in_maps.append({
            "src": src16,
            "tgt": tgt16,
            "x1c": np.ascontiguousarray(src16[:, :, sl]),
            "x2c": np.ascontiguousarray(tgt16[:, :, sl]),
            "wq": b16(np.asarray(Wq)[f_list, :].T),
            "wk": b16(np.asarray(Wk)[f_list, :].T),
            "wv": b16(np.asarray(Wv)[f_list, :].T),
            "bq": np.ascontiguousarray(
                np.asarray(bq)[f_list].reshape(D, 1), np.float32),
            "bv": np.ascontiguousarray(
                np.asarray(bv)[f_list].reshape(D, 1), np.float32),
            "wmT": wm_perm,
            "bm": bm_c,
            "w1T": w1t,
            "w2T": w2t,
            "b2": b2_c,
            "wk2T": wk2t,
            "wv2T": wv2t,
        })
    return in_maps


def kernel(**inputs):
    if "nc" not in _CACHE:
        _CACHE["nc"] = build_nc()
    nc = _CACHE["nc"]
    in_maps = _prep_inputs(**inputs)
    res = run_bass_kernel_spmd(nc, in_maps, list(range(NCORES)))
    src_out = np.concatenate(
        [res.results[c]["src_out_c"] for c in range(NCORES)], axis=2)
    tgt_out = np.concatenate(
        [res.results[c]["tgt_out_c"] for c in range(NCORES)], axis=2)
    return (src_out, tgt_out)
